# revision 15
# baseline (speedup 1.0000x reference)
"""Trainium2 Bass kernel for nn_D2IM_Net (D2IM losses).

Self-contained: takes FULL inputs as numpy arrays, shards batch (B=32) over
8 NeuronCores (4 samples each), runs one Bass/Tile kernel SPMD, and reduces
per-partition accumulator columns on the host (f64) into the 3 scalar losses.

Per-core pipeline (4 samples):
  - transmat/scale scalars broadcast to all partitions via a ones-matmul (PE)
  - point projection via fused scalar_tensor_tensor chains (DVE)
  - exact floor/clamp of pixel coords with the 2^23 magic-number trick
    (rounding-mode independent)
  - 3x3 finite-difference stencils: x-shifts via free-dim APs; y-shifts via
    DMA-shifted image copies (engine APs need 32-aligned partition bases, so
    cross-partition reads are done by DMA, never by compute APs)
  - the 4 gather maps (pdf, pdb, gtlap G, predlap P) stored as bf16
    full-image tables, one partition per (sample, idx-half, map); gathered
    with gpsimd ap_gather (d=2 pairs, k = idx>>1), parity-selected afterwards
  - k-index streams wrapped into ap_gather's mod-16 partition layout with
    16 PE transposes per sample into PSUM + one ACT f32->int16 convert +
    2 small DMAs
  - losses via fused ops with accum_out; the [128, 32] accumulator tile is
    DMAd out and reduced on the host in f64.
"""

import os
import sys

import numpy as np

for _p in ("/opt/trn_rl_repo", "/root/.axon_site/_ro/trn_rl_repo"):
    if os.path.isdir(_p) and _p not in sys.path:
        sys.path.insert(0, _p)

import concourse.bacc as bacc
import concourse.mybir as mybir
import concourse.tile as tile
from concourse.bass_utils import run_bass_kernel_spmd

dt = mybir.dt
Alu = mybir.AluOpType

B, N, RES = 32, 32768, 224
NCORES = 8
SPC = B // NCORES            # samples per core = 4
NPIX = RES * RES             # 50176
NE = NPIX // 2               # 25088 bf16 pairs per table
NI = N // 2                  # 16384 idxs per (sample, half) group
NCALL = 2                    # gather split into NCALL calls
NIC = NI // NCALL            # idxs per call per group
MAGIC = 8388608.0            # 2^23
DEBUG = os.environ.get("D2IM_DEBUG") == "1"

_cache = {}


def _f32(x):
    return np.ascontiguousarray(x, dtype=np.float32)


def build_nc():
    nc = bacc.Bacc("TRN2", target_bir_lowering=False, debug=False,
                   enable_asserts=False, num_devices=NCORES)

    pts_d = nc.dram_tensor("pts", [SPC, N, 3], dt.float32, kind="ExternalInput")
    grd_d = nc.dram_tensor("grd", [SPC, N, 3], dt.float32, kind="ExternalInput")
    gtv_d = nc.dram_tensor("gtv", [SPC, N], dt.float32, kind="ExternalInput")
    bsv_d = nc.dram_tensor("bsv", [SPC, N], dt.float32, kind="ExternalInput")
    pdp_d = nc.dram_tensor("pdp", [SPC, 2, RES, RES], dt.float32,
                           kind="ExternalInput")
    mc_d = nc.dram_tensor("mc45", [SPC, 2, RES, RES], dt.float32,
                          kind="ExternalInput")
    tms_d = nc.dram_tensor("tms", [1, 49], dt.float32, kind="ExternalInput")
    acc_d = nc.dram_tensor("acc", [128, 32], dt.float32, kind="ExternalOutput")
    if DEBUG:
        dbg_kf = nc.dram_tensor("dbg_kf", [SPC, 128, 256], dt.float32,
                                kind="ExternalOutput")
        dbg_idx = nc.dram_tensor("dbg_idx", [128, 1024], dt.int16,
                                 kind="ExternalOutput")
        dbg_gout = nc.dram_tensor("dbg_gout", [NCALL, 128, 2 * NIC],
                                  dt.bfloat16, kind="ExternalOutput")
        dbg_pairs = nc.dram_tensor("dbg_pairs", [128, 8192], dt.bfloat16,
                                   kind="ExternalOutput")
        dbg_tabs = nc.dram_tensor("dbg_tabs", [128, NPIX], dt.bfloat16,
                                  kind="ExternalOutput")

    with tile.TileContext(nc) as tc:
        with tc.tile_pool(name="const", bufs=1) as cpool, \
             tc.tile_pool(name="ptsp", bufs=1) as ptspool, \
             tc.tile_pool(name="valp", bufs=1) as valpool, \
             tc.tile_pool(name="payp", bufs=4) as paypool, \
             tc.tile_pool(name="imgp", bufs=1) as imgpool, \
             tc.tile_pool(name="stnp", bufs=1) as stnpool, \
             tc.tile_pool(name="mbfp", bufs=1) as mbfpool, \
             tc.tile_pool(name="kfp", bufs=2) as kfpool, \
             tc.tile_pool(name="bigp", bufs=1) as bigpool, \
             tc.tile_pool(name="gop", bufs=1) as gopool, \
             tc.tile_pool(name="tmpp", bufs=1) as tmppool, \
             tc.tile_pool(name="psp", bufs=1, space="PSUM") as pspool:

            # ---------- constants ----------
            ident = cpool.tile([128, 128], dt.float32, tag="ident")
            pidx = cpool.tile([128, 1], dt.int32, tag="pidx")
            icol_t = tmppool.tile([128, 256], dt.float32, tag="t1",
                                  name="icol_t")
            icol = icol_t[:, 0:128].bitcast(dt.int32)
            nc.gpsimd.iota(icol, [[1, 128]], base=0, channel_multiplier=0)
            nc.gpsimd.iota(pidx[:], [[0, 1]], base=0, channel_multiplier=1)
            nc.vector.tensor_tensor(ident[:], icol,
                                    pidx[:].broadcast_to((128, 128)),
                                    Alu.is_equal)

            stage = cpool.tile([1, 49], dt.float32, tag="stage")
            nc.sync.dma_start(stage[:], tms_d.ap()[:])
            ones1 = cpool.tile([1, 128], dt.float32, tag="ones1")
            nc.vector.memset(ones1[:], 1.0)
            bc_ps = pspool.tile([128, 64], dt.float32, tag="bcps")
            nc.tensor.matmul(bc_ps[:, 0:49], ones1[:], stage[:],
                             start=True, stop=True)
            scal = cpool.tile([128, 64], dt.float32, tag="scal")
            nc.scalar.copy(scal[:, 0:49], bc_ps[:, 0:49])
            # col 49: c2 = 2*49*scale/4 = 24.5*scale  (host divides sq-sum by 4)
            nc.vector.tensor_scalar(scal[:, 49:50], scal[:, 48:49], 24.5, None,
                                    Alu.mult)

            acc = cpool.tile([128, 32], dt.float32, tag="acc")
            nc.vector.memset(acc[:], 0.0)

            # gather tables: one partition per (s, h, m): partition 32s+16h+m
            tabs = bigpool.tile([128, NPIX], dt.bfloat16, tag="tabs")
            # wrapped int16 k-indices: partition 32s+16h+r, slot q
            idxt = cpool.tile([128, NI // 16], dt.int16, tag="idxt")
            # int16 staging before the 16-partition-offset fixup DMA
            stg16 = cpool.tile([128, 2048], dt.int16, tag="stg16")

            ROW0 = slice(2, 226)     # data cols of row r=0 in padded imgs
            ROW1 = slice(228, 452)

            def load_img(name, dram_ap, shifted):
                """[112, 452] tile; partition p = rows (2p, 2p+1), each row at
                2 + 226*r with 2 zero pad cols before it. shifted=True loads
                rows (2p+2, 2p+3) instead (for y+1/y+2 stencil taps)."""
                t = imgpool.tile([112, 452], dt.float32, tag=name, name=name)
                if shifted:
                    nc.gpsimd.memset(t[:], 0.0)
                    nc.sync.dma_start(
                        t[0:111, :].rearrange("p (r c) -> p r c", r=2)
                        [:, :, 2:226],
                        dram_ap[2:224].rearrange("(p r) c -> p r c", r=2))
                else:
                    nc.gpsimd.memset(
                        t[:].rearrange("p (r c) -> p r c", r=2)[:, :, 0:2], 0.0)
                    nc.sync.dma_start(
                        t[:].rearrange("p (r c) -> p r c", r=2)[:, :, 2:226],
                        dram_ap.rearrange("(p r) c -> p r c", r=2))
                return t

            # ---------- per-sample pre-gather ----------
            fw_t, lw_t, d_t, r_t, par_t = {}, {}, {}, {}, {}
            for s in range(SPC):
                a0 = 8 * s  # accumulator column base

                pts = ptspool.tile([128, 768], dt.float32, tag="pts")
                nc.sync.dma_start(
                    pts[:].rearrange("p (n c) -> p n c", c=3),
                    pts_d.ap()[s].rearrange("(p n) c -> p n c", p=128))
                grd = ptspool.tile([128, 768], dt.float32, tag="grd")
                nc.sync.dma_start(
                    grd[:].rearrange("p (n c) -> p n c", c=3),
                    grd_d.ap()[s].rearrange("(p n) c -> p n c", p=128))
                gtv = valpool.tile([128, 256], dt.float32, tag="gtv", bufs=2)
                nc.sync.dma_start(gtv[:],
                                  gtv_d.ap()[s].rearrange("(p n) -> p n", p=128))
                bsv = valpool.tile([128, 256], dt.float32, tag="bsv")
                nc.sync.dma_start(bsv[:],
                                  bsv_d.ap()[s].rearrange("(p n) -> p n", p=128))

                pv = pts[:].rearrange("p (n c) -> p c n", c=3)
                gv = grd[:].rearrange("p (n c) -> p c n", c=3)

                def sc(f):
                    return scal[:, 12 * s + f:12 * s + f + 1]

                def tmp(tag):
                    return tmppool.tile([128, 256], dt.float32, tag=tag,
                                        name=f"{tag}_{s}")

                t1, t2, t3 = tmp("t1"), tmp("t2"), tmp("t3")

                # projection: col j: sum_i coord_i*T[i][j] + T[3][j]
                xp, yp, zp = tmp("xp"), tmp("yp"), tmp("zp")
                for (dst, j) in ((xp, 0), (yp, 1), (zp, 2)):
                    nc.vector.tensor_scalar(t1[:], pv[:, 0], sc(0 + j),
                                            sc(9 + j), Alu.mult, Alu.add)
                    nc.vector.scalar_tensor_tensor(t2[:], pv[:, 1], sc(3 + j),
                                                   t1[:], Alu.mult, Alu.add)
                    nc.vector.scalar_tensor_tensor(dst[:], pv[:, 2], sc(6 + j),
                                                   t2[:], Alu.mult, Alu.add)
                # grad z-projection (homogeneous pad is 0 -> no T[3][2] term)
                nc.vector.tensor_scalar(t1[:], gv[:, 0], sc(2), None, Alu.mult)
                nc.vector.scalar_tensor_tensor(t2[:], gv[:, 1], sc(5), t1[:],
                                               Alu.mult, Alu.add)
                nc.vector.scalar_tensor_tensor(t3[:], gv[:, 2], sc(8), t2[:],
                                               Alu.mult, Alu.add)
                # fw = 0.5 - 0.5*sign(gz) = (gz<0) + 0.5*(gz==0)
                nc.vector.tensor_scalar(t1[:], t3[:], 0.0, None, Alu.is_lt)
                nc.vector.tensor_scalar(t2[:], t3[:], 0.0, None, Alu.is_equal)
                fw = paypool.tile([128, 256], dt.bfloat16, tag="fw")
                nc.vector.scalar_tensor_tensor(fw[:], t2[:], 0.5, t1[:],
                                               Alu.mult, Alu.add,
                                               accum_out=acc[:, a0 + 1:a0 + 2])
                fw_t[s] = fw

                rr = paypool.tile([128, 256], dt.float32, tag="rr")
                nc.vector.reciprocal(rr[:], zp[:])
                r_t[s] = rr

                # pixel coords: jax's astype(int32) here rounds to nearest
                # even, which is exactly what the f32 +2^23 trick does
                def roundclamp(t):
                    nc.vector.tensor_tensor(t1[:], t[:], rr[:], Alu.mult)
                    nc.vector.tensor_scalar(t2[:], t1[:], 224.0, MAGIC,
                                            Alu.min, Alu.add)
                    nc.vector.tensor_scalar(t[:], t2[:], MAGIC, 223.0,
                                            Alu.subtract, Alu.min)

                roundclamp(xp)   # xp <- clip(rint(xp/zp), 0, 223)
                roundclamp(yp)

                # int path: xi = int(xf); par = xi & 1 (u8 mask);
                # k = (xi >> 1) + 112*yf
                xi = t1[:].bitcast(dt.int32)
                nc.vector.tensor_scalar(xi, xp[:], 0, None, Alu.bypass)
                pari = t3[:].bitcast(dt.int32)
                nc.vector.tensor_scalar(pari, xi, 1, None, Alu.bitwise_and)
                par = paypool.tile([128, 256], dt.uint8, tag="par")
                nc.vector.tensor_copy(par[:], pari)
                par_t[s] = par
                xh = t2[:].bitcast(dt.int32)
                nc.vector.tensor_scalar(xh, xi, 1, None, Alu.arith_shift_right)
                kf = kfpool.tile([128, 256], dt.float32, tag="kf")
                nc.vector.scalar_tensor_tensor(kf[:], yp[:], 112.0, xh,
                                               Alu.mult, Alu.add)
                if DEBUG:
                    nc.sync.dma_start(dbg_kf.ap()[s], kf[:])

                # wrap k: 16 PE transposes -> PSUM[0:16, (u,p) @128u+p]
                ktr = pspool.tile([16, 2048], dt.float32, tag="ktr",
                                  name=f"ktr_{s}")
                for u in range(16):
                    nc.tensor.transpose(
                        ktr[:, 128 * u:128 * u + 128],
                        kf[:, 16 * u:16 * u + 16],
                        ident[:], tile_position=(0, 0))
                # one ACT convert: stg16[32s+r, 1024h+q] = k for stream
                # (h, t=16q+r); in cols 128u + 64h + p'
                nc.scalar.copy(
                    stg16[32 * s:32 * s + 16, :],
                    ktr[:].rearrange("p (u h a) -> p h a u", u=16, h=2))
                # fixup DMAs to the odd-16 group partitions
                for h in range(2):
                    nc.scalar.dma_start(
                        idxt[32 * s + 16 * h:32 * s + 16 * h + 16, :],
                        stg16[32 * s:32 * s + 16,
                              1024 * h:1024 * h + 1024])

                # d = base - gt ; base-loss accumulation ; lap mask
                d = paypool.tile([128, 256], dt.float32, tag="d")
                nc.vector.tensor_tensor(d[:], bsv[:], gtv[:], Alu.subtract)
                d_t[s] = d
                nc.vector.scalar_tensor_tensor(t1[:], d[:], 1.0, d[:],
                                               Alu.mult, Alu.mult,
                                               accum_out=acc[:, a0:a0 + 1])
                nc.vector.tensor_scalar(t2[:], gtv[:], 0.1, None, Alu.is_lt)
                lw = paypool.tile([128, 256], dt.bfloat16, tag="lw")
                nc.vector.scalar_tensor_tensor(lw[:], t2[:], 1.0, fw[:],
                                               Alu.mult, Alu.mult,
                                               accum_out=acc[:, a0 + 4:a0 + 5])
                lw_t[s] = lw

                # ---------- images + stencils + bf16 map tiles ----------
                mapbf = {m: mbfpool.tile([112, 448], dt.bfloat16, tag=f"mb{m}",
                                         name=f"mapbf{m}_{s}", bufs=2)
                         for m in range(4)}

                imgF = load_img("imgF", pdp_d.ap()[s, 0], False)
                fshF = load_img("fshF", pdp_d.ap()[s, 0], True)
                nc.vector.tensor_copy(
                    mapbf[0][:],
                    imgF[:].rearrange("p (r c) -> p r c", r=2)[:, :, 2:226])
                imgB = load_img("imgB", pdp_d.ap()[s, 1], False)
                nc.vector.tensor_copy(
                    mapbf[1][:],
                    imgB[:].rearrange("p (r c) -> p r c", r=2)[:, :, 2:226])

                fr = imgF[:].rearrange("p (r c) -> p r c", r=2)
                f_x0 = fr[:, :, 2:226]
                f_x1 = fr[:, :, 1:225]
                f_x2 = fr[:, :, 0:224]
                # nx2 = f - 2*f(x-1) + f(x-2)
                nx2 = stnpool.tile([112, 448], dt.float32, tag="nx2")
                nxv = nx2[:].rearrange("p (r c) -> p r c", r=2)
                nc.vector.scalar_tensor_tensor(nxv, f_x1, -2.0, f_x0,
                                               Alu.mult, Alu.add)
                nc.vector.tensor_tensor(nxv, nxv, f_x2, Alu.add)
                # ny2 = f - 2*f(y+1) + f(y+2) using the shifted copy fshF
                ny2 = stnpool.tile([112, 448], dt.float32, tag="ny2")
                f0, f1 = imgF[:, ROW0], imgF[:, ROW1]
                s0, s1 = fshF[:, ROW0], fshF[:, ROW1]
                nc.vector.scalar_tensor_tensor(ny2[:, 0:224], f1, -2.0, f0,
                                               Alu.mult, Alu.add)
                nc.vector.tensor_tensor(ny2[:, 0:224], ny2[:, 0:224], s0,
                                        Alu.add)
                nc.vector.scalar_tensor_tensor(ny2[:, 224:448], s0, -2.0, f1,
                                               Alu.mult, Alu.add)
                nc.vector.tensor_tensor(ny2[:, 224:448], ny2[:, 224:448], s1,
                                        Alu.add)
                # P (unhalved) -> bf16, even/odd row-halves interleaved back
                nc.vector.tensor_tensor(
                    mapbf[3][:].rearrange("p (r c) -> p r c", r=2)[:, 0],
                    nx2[:].rearrange("p (r c) -> p r c", r=2)[:, 0],
                    ny2[:, 0:224], Alu.add)
                nc.vector.tensor_tensor(
                    mapbf[3][:].rearrange("p (r c) -> p r c", r=2)[:, 1],
                    nx2[:].rearrange("p (r c) -> p r c", r=2)[:, 1],
                    ny2[:, 224:448], Alu.add)

                # G (unhalved): (mc5 - mc5(x-1)) + (mc4 - mc4(y+1))
                img4 = load_img("img4", mc_d.ap()[s, 0], False)
                fsh4 = load_img("fsh4", mc_d.ap()[s, 0], True)
                img5 = load_img("img5", mc_d.ap()[s, 1], False)
                g5 = img5[:].rearrange("p (r c) -> p r c", r=2)
                g1 = stnpool.tile([112, 448], dt.float32, tag="g1")
                g1v = g1[:].rearrange("p (r c) -> p r c", r=2)
                nc.vector.scalar_tensor_tensor(g1v, g5[:, :, 1:225], -1.0,
                                               g5[:, :, 2:226], Alu.mult,
                                               Alu.add)
                q0, q1 = img4[:, ROW0], img4[:, ROW1]
                w0 = fsh4[:, ROW0]
                ve = stnpool.tile([112, 448], dt.float32, tag="nx2",
                                  name=f"ve_{s}")
                nc.vector.tensor_tensor(ve[:, 0:224], q0, q1, Alu.subtract)
                nc.vector.tensor_tensor(ve[:, 224:448], q1, w0, Alu.subtract)
                nc.vector.tensor_tensor(
                    mapbf[2][:].rearrange("p (r c) -> p r c", r=2)[:, 0],
                    g1[:].rearrange("p (r c) -> p r c", r=2)[:, 0],
                    ve[:, 0:224], Alu.add)
                nc.vector.tensor_tensor(
                    mapbf[2][:].rearrange("p (r c) -> p r c", r=2)[:, 1],
                    g1[:].rearrange("p (r c) -> p r c", r=2)[:, 1],
                    ve[:, 224:448], Alu.add)

                # table DMAs: partition 32s+16h+m <- mapbf[m]
                for m in range(4):
                    for h in range(2):
                        p0 = 32 * s + 16 * h + m
                        eng = nc.sync if (m + h) % 2 == 0 else nc.scalar
                        eng.dma_start(
                            tabs[p0:p0 + 1, :]
                            .rearrange("p (a c) -> p a c", a=112),
                            mapbf[m][:].unsqueeze(1))

            if DEBUG:
                nc.sync.dma_start(dbg_idx.ap()[:], idxt[:])
                nc.sync.dma_start(dbg_tabs.ap()[:], tabs[:])
            # ---------- gathers + repacks ----------
            pairs = bigpool.tile([128, 16 * 512], dt.bfloat16, tag="pairs")
            for call in range(NCALL):
                gout = gopool.tile([128, 2 * NIC], dt.bfloat16, tag="gout",
                                   name=f"gout{call}")
                nc.gpsimd.ap_gather(
                    gout[:].rearrange("p (k e) -> p k e", e=2),
                    tabs[:].rearrange("p (k e) -> p k e", e=2),
                    idxt[:, 512 * call:512 * call + 512],
                    channels=128, num_elems=NE, d=2, num_idxs=NIC)
                if DEBUG:
                    nc.sync.dma_start(dbg_gout.ap()[call], gout[:])
                # stream positions t in [NIC*call, ...): points n = 16384h + t
                # -> partitions [64h + 32*call, +32), 512 pair-cols per block
                npart = NIC // 256
                for s in range(SPC):
                    for m in range(4):
                        blk = (4 * s + m) * 512
                        for h in range(2):
                            p0 = 64 * h + npart * call
                            eng = (nc.scalar, nc.sync)[(m + h) % 2]
                            eng.dma_start(
                                pairs[p0:p0 + npart, blk:blk + 512]
                                .unsqueeze(1),
                                gout[32 * s + 16 * h + m:
                                     32 * s + 16 * h + m + 1, :]
                                .rearrange("p (a c) -> p a c", a=npart))

            if DEBUG:
                nc.sync.dma_start(dbg_pairs.ap()[:], pairs[:])
            # ---------- select + losses ----------
            for s in range(SPC):
                a0 = 8 * s
                fw, lw, d, rr, par = (fw_t[s], lw_t[s], d_t[s], r_t[s],
                                      par_t[s])
                sel = {}
                for m in range(4):
                    blk = (4 * s + m) * 512
                    pv2 = pairs[:, blk:blk + 512].rearrange(
                        "p (n e) -> p n e", e=2)
                    nc.vector.copy_predicated(pv2[:, :, 0], par[:],
                                              pv2[:, :, 1])
                    sel[m] = pv2[:, :, 0]

                def tmp(tag):
                    return tmppool.tile([128, 256], dt.float32, tag=tag,
                                        name=f"{tag}_post{s}")

                t1, t2, t3, t4, jk = (tmp("t1"), tmp("t2"), tmp("t3"),
                                      tmp("yp"), tmp("xp"))

                # front/back weighted L1
                nc.vector.tensor_tensor(t1[:], d[:], sel[0], Alu.add)
                nc.scalar.activation(t1[:], t1[:],
                                     mybir.ActivationFunctionType.Abs)
                nc.vector.scalar_tensor_tensor(
                    jk[:], t1[:], 1.0, fw[:], Alu.mult, Alu.mult,
                    accum_out=acc[:, a0 + 2:a0 + 3])
                nc.vector.tensor_scalar(t2[:], fw[:], -1.0, 1.0, Alu.mult,
                                        Alu.add)
                nc.vector.tensor_tensor(t3[:], d[:], sel[1], Alu.add)
                nc.scalar.activation(t3[:], t3[:],
                                     mybir.ActivationFunctionType.Abs)
                nc.vector.scalar_tensor_tensor(
                    jk[:], t3[:], 1.0, t2[:], Alu.mult, Alu.mult,
                    accum_out=acc[:, a0 + 3:a0 + 4])
                # laplacian: s' = gP*(24.5*scale)/z + gG  (= 2x ref's value;
                # host divides the squared sum by 4)
                nc.vector.tensor_tensor(t4[:], sel[3], rr[:], Alu.mult)
                nc.vector.scalar_tensor_tensor(t1[:], t4[:], scal[:, 49:50],
                                               sel[2], Alu.mult, Alu.add)
                nc.scalar.square(t2[:], t1[:])
                nc.vector.scalar_tensor_tensor(
                    jk[:], t2[:], 1.0, lw[:], Alu.mult, Alu.mult,
                    accum_out=acc[:, a0 + 5:a0 + 6])

            nc.sync.dma_start(acc_d.ap()[:], acc[:])

    nc.compile()
    return nc


def _get_nc():
    if "nc" not in _cache:
        _cache["nc"] = build_nc()
    return _cache["nc"]


def kernel(gt_points, gt_values, gt_gradients, mc_image, gt_transmat, scale,
           base_values, pred_disp):
    nc = _get_nc()
    gt_points = _f32(gt_points)
    gt_values = _f32(gt_values)
    gt_gradients = _f32(gt_gradients)
    mc45 = _f32(mc_image[:, 4:6])
    gt_transmat = _f32(gt_transmat)
    scale = _f32(scale)
    base_values = _f32(base_values)
    pred_disp = _f32(pred_disp)

    in_maps = []
    for c in range(NCORES):
        sl = slice(SPC * c, SPC * (c + 1))
        tms = np.zeros((1, 49), np.float32)
        tms[0, :48] = gt_transmat[sl].reshape(-1)
        tms[0, 48] = scale[0]
        in_maps.append({
            "pts": _f32(gt_points[sl]),
            "grd": _f32(gt_gradients[sl]),
            "gtv": _f32(gt_values[sl, :, 0]),
            "bsv": _f32(base_values[sl, :, 0]),
            "pdp": _f32(pred_disp[sl]),
            "mc45": _f32(mc45[sl]),
            "tms": tms,
        })

    res = run_bass_kernel_spmd(nc, in_maps, core_ids=list(range(NCORES)))

    sq = fwsum = fa = ba = lwsum = tm = 0.0
    for c in range(NCORES):
        a = res.results[c]["acc"].astype(np.float64)
        for s in range(SPC):
            a0 = 8 * s
            sq += a[:, a0 + 0].sum()
            fwsum += a[:, a0 + 1].sum()
            fa += a[:, a0 + 2].sum()
            ba += a[:, a0 + 3].sum()
            lwsum += a[:, a0 + 4].sum()
            tm += a[:, a0 + 5].sum()

    tot = float(B * N)
    loss_base = sq / tot
    loss_front = fa / fwsum
    loss_back = ba / (tot - fwsum)
    loss_sdf = 0.5 * (loss_front + loss_back)
    loss_lap = tm / (4.0 * lwsum)
    return np.array([loss_base, loss_sdf, loss_lap], dtype=np.float32)


# revision 19
# speedup vs baseline: 1.0677x; 1.0677x over previous
"""Trainium2 Bass kernel for nn_D2IM_Net (D2IM losses).

Self-contained: takes FULL inputs as numpy arrays, shards batch (B=32) over
8 NeuronCores (4 samples each), runs one Bass/Tile kernel SPMD, and reduces
per-partition accumulator columns on the host (f64) into the 3 scalar losses.

Per-core pipeline (4 samples):
  - transmat/scale scalars broadcast to all partitions via a ones-matmul (PE)
  - point projection via fused scalar_tensor_tensor chains (DVE, 2x mode)
  - pixel coords: jax's astype(int32) rounds to nearest-even on this path,
    reproduced exactly by the f32 +2^23 magic add
  - 3x3 finite-difference stencils: x-shifts via free-dim APs; y-shifts via
    DMA-shifted image copies (engine APs need 32-aligned partition bases)
  - the 4 gather maps (pdf, pdb, gtlap G, predlap P) stored as bf16
    full-image tables, one partition per (sample, idx-half, map); gathered
    with gpsimd ap_gather (d=2 pairs, k = idx>>1), parity-selected afterwards
  - k-index streams wrapped into ap_gather's mod-16 partition layout with
    16 PE transposes per sample into PSUM + one ACT f32->int16 convert +
    2 small fixup DMAs
  - per-point payloads (d, fw, lw, par, 1/z) live in 4-sample-wide
    [128, 1024] tiles so the post-gather select + loss phase runs as a
    handful of wide fused ops with accum_out columns
  - the [128, 8] accumulator tile is DMAd out and reduced on the host (f64)
"""

import os
import sys

import numpy as np

for _p in ("/opt/trn_rl_repo", "/root/.axon_site/_ro/trn_rl_repo"):
    if os.path.isdir(_p) and _p not in sys.path:
        sys.path.insert(0, _p)

import concourse.bacc as bacc
import concourse.mybir as mybir
import concourse.tile as tile
from concourse.bass_utils import run_bass_kernel_spmd

dt = mybir.dt
Alu = mybir.AluOpType
Act = mybir.ActivationFunctionType

B, N, RES = 32, 32768, 224
NCORES = 8
SPC = B // NCORES            # samples per core = 4
NPIX = RES * RES             # 50176
NE = NPIX // 2               # 25088 bf16 pairs per table
NI = N // 2                  # 16384 idxs per (sample, half) group
NCALL = 2                    # gather split into NCALL calls
NIC = NI // NCALL            # idxs per call per group
MAGIC = 8388608.0            # 2^23
DEBUG = os.environ.get("D2IM_DEBUG") == "1"

_cache = {}


def _f32(x):
    return np.ascontiguousarray(x, dtype=np.float32)


def build_nc():
    nc = bacc.Bacc("TRN2", target_bir_lowering=False, debug=False,
                   enable_asserts=False, num_devices=NCORES)

    pts_d = nc.dram_tensor("pts", [SPC, N, 3], dt.float32, kind="ExternalInput")
    grd_d = nc.dram_tensor("grd", [SPC, N, 3], dt.float32, kind="ExternalInput")
    gtv_d = nc.dram_tensor("gtv", [SPC, N], dt.float32, kind="ExternalInput")
    bsv_d = nc.dram_tensor("bsv", [SPC, N], dt.float32, kind="ExternalInput")
    pdp_d = nc.dram_tensor("pdp", [SPC, 2, RES, RES], dt.float32,
                           kind="ExternalInput")
    mc_d = nc.dram_tensor("mc45", [SPC, 2, RES, RES], dt.float32,
                          kind="ExternalInput")
    tms_d = nc.dram_tensor("tms", [1, 49], dt.float32, kind="ExternalInput")
    acc_d = nc.dram_tensor("acc", [128, 8], dt.float32, kind="ExternalOutput")
    if DEBUG:
        dbg_kf = nc.dram_tensor("dbg_kf", [SPC, 128, 256], dt.float32,
                                kind="ExternalOutput")
        dbg_idx = nc.dram_tensor("dbg_idx", [128, 1024], dt.int16,
                                 kind="ExternalOutput")
        dbg_pairs = nc.dram_tensor("dbg_pairs", [128, 8192], dt.bfloat16,
                                   kind="ExternalOutput")
        dbg_tabs = nc.dram_tensor("dbg_tabs", [128, NPIX], dt.bfloat16,
                                  kind="ExternalOutput")

    with tile.TileContext(nc) as tc:
        with tc.tile_pool(name="const", bufs=1) as cpool, \
             tc.tile_pool(name="ptsp", bufs=1) as ptspool, \
             tc.tile_pool(name="valp", bufs=1) as valpool, \
             tc.tile_pool(name="payp", bufs=1) as paypool, \
             tc.tile_pool(name="imgp", bufs=1) as imgpool, \
             tc.tile_pool(name="stnp", bufs=1) as stnpool, \
             tc.tile_pool(name="mbfp", bufs=1) as mbfpool, \
             tc.tile_pool(name="kfp", bufs=1) as kfpool, \
             tc.tile_pool(name="bigp", bufs=1) as bigpool, \
             tc.tile_pool(name="gop", bufs=1) as gopool, \
             tc.tile_pool(name="tmpp", bufs=1) as tmppool, \
             tc.tile_pool(name="psp", bufs=1, space="PSUM") as pspool:

            # ---------- constants ----------
            ident = cpool.tile([128, 128], dt.float32, tag="ident")
            pidx = cpool.tile([128, 1], dt.int32, tag="pidx")
            icol_t = tmppool.tile([128, 256], dt.float32, tag="t1",
                                  name="icol_t")
            icol = icol_t[:, 0:128].bitcast(dt.int32)
            nc.gpsimd.iota(icol, [[1, 128]], base=0, channel_multiplier=0)
            nc.gpsimd.iota(pidx[:], [[0, 1]], base=0, channel_multiplier=1)
            nc.vector.tensor_tensor(ident[:], icol,
                                    pidx[:].broadcast_to((128, 128)),
                                    Alu.is_equal)

            stage = cpool.tile([1, 49], dt.float32, tag="stage")
            nc.sync.dma_start(stage[:], tms_d.ap()[:])
            ones1 = cpool.tile([1, 128], dt.float32, tag="ones1")
            nc.vector.memset(ones1[:], 1.0)
            bc_ps = pspool.tile([128, 64], dt.float32, tag="bcps")
            nc.tensor.matmul(bc_ps[:, 0:49], ones1[:], stage[:],
                             start=True, stop=True)
            scal = cpool.tile([128, 64], dt.float32, tag="scal")
            nc.scalar.copy(scal[:, 0:49], bc_ps[:, 0:49])
            # col 49: c2 = 2*49*scale/4 = 24.5*scale  (host divides sq-sum by 4)
            nc.vector.tensor_scalar(scal[:, 49:50], scal[:, 48:49], 24.5, None,
                                    Alu.mult)

            acc = cpool.tile([128, 8], dt.float32, tag="acc")
            nc.vector.memset(acc[:], 0.0)

            # gather tables: one partition per (s, h, m): partition 32s+16h+m
            tabs = bigpool.tile([128, NPIX], dt.bfloat16, tag="tabs")
            # wrapped int16 k-indices: partition 32s+16h+r, slot q
            idxt = cpool.tile([128, NI // 16], dt.int16, tag="idxt")
            # int16 staging before the 16-partition-offset fixup DMA
            stg16 = cpool.tile([128, 2048], dt.int16, tag="stg16")

            # 4-sample-wide payload tiles; col = 256*s + c for point (p, c)
            d_b = paypool.tile([128, 1024], dt.float32, tag="d")
            rr_b = paypool.tile([128, 1024], dt.float32, tag="rr")
            fw_b = paypool.tile([128, 1024], dt.bfloat16, tag="fw")
            lw_b = paypool.tile([128, 1024], dt.bfloat16, tag="lw")
            par_b = paypool.tile([128, 1024], dt.uint8, tag="par")

            ROW0 = slice(2, 226)     # data cols of row r=0 in padded imgs
            ROW1 = slice(228, 452)

            def load_img(name, dram_ap, shifted):
                """[112, 452] tile; partition p = rows (2p, 2p+1), each row at
                2 + 226*r with 2 zero pad cols. shifted=True: rows (2p+2,
                2p+3) for the y+1/y+2 taps; rows >= 224 are zeros."""
                t = imgpool.tile([112, 452], dt.float32, tag=name, name=name)
                if shifted:
                    nc.gpsimd.memset(t[96:112, :], 0.0)
                    nc.gpsimd.memset(
                        t[:].rearrange("p (r c) -> p r c", r=2)[:, :, 0:2],
                        0.0)
                    nc.sync.dma_start(
                        t[0:111, :].rearrange("p (r c) -> p r c", r=2)
                        [:, :, 2:226],
                        dram_ap[2:224].rearrange("(p r) c -> p r c", r=2))
                else:
                    nc.gpsimd.memset(
                        t[:].rearrange("p (r c) -> p r c", r=2)[:, :, 0:2],
                        0.0)
                    nc.sync.dma_start(
                        t[:].rearrange("p (r c) -> p r c", r=2)[:, :, 2:226],
                        dram_ap.rearrange("(p r) c -> p r c", r=2))
                return t

            def stt(out, in0, scalar, in1, op0, op1, accum=None):
                nc.vector.scalar_tensor_tensor(out, in0, scalar, in1, op0,
                                               op1, accum_out=accum)

            # ---------- per-sample pre-gather ----------
            for s in range(SPC):
                S = slice(256 * s, 256 * s + 256)

                pts = ptspool.tile([128, 768], dt.float32, tag="pts")
                nc.sync.dma_start(
                    pts[:].rearrange("p (n c) -> p n c", c=3),
                    pts_d.ap()[s].rearrange("(p n) c -> p n c", p=128))
                grd = ptspool.tile([128, 768], dt.float32, tag="grd")
                nc.sync.dma_start(
                    grd[:].rearrange("p (n c) -> p n c", c=3),
                    grd_d.ap()[s].rearrange("(p n) c -> p n c", p=128))
                gtv = valpool.tile([128, 256], dt.float32, tag="gtv", bufs=2)
                nc.sync.dma_start(
                    gtv[:], gtv_d.ap()[s].rearrange("(p n) -> p n", p=128))
                bsv = valpool.tile([128, 256], dt.float32, tag="bsv", bufs=2)
                nc.sync.dma_start(
                    bsv[:], bsv_d.ap()[s].rearrange("(p n) -> p n", p=128))

                pv = pts[:].rearrange("p (n c) -> p c n", c=3)
                gv = grd[:].rearrange("p (n c) -> p c n", c=3)

                def sc(f):
                    return scal[:, 12 * s + f:12 * s + f + 1]

                def tmp(tag):
                    return tmppool.tile([128, 256], dt.float32, tag=tag,
                                        name=f"{tag}_{s}")

                t1, t2, t3 = tmp("t1"), tmp("t2"), tmp("t3")
                xp, yp, zp = tmp("xp"), tmp("yp"), tmp("zp")

                # projection: col j: sum_i coord_i*T[i][j] + T[3][j]
                for (dst, j) in ((xp, 0), (yp, 1), (zp, 2)):
                    nc.vector.tensor_scalar(t1[:], pv[:, 0], sc(0 + j),
                                            sc(9 + j), Alu.mult, Alu.add)
                    stt(t2[:], pv[:, 1], sc(3 + j), t1[:], Alu.mult, Alu.add)
                    stt(dst[:], pv[:, 2], sc(6 + j), t2[:], Alu.mult, Alu.add)
                # grad z-projection (homogeneous pad is 0)
                nc.vector.tensor_scalar(t1[:], gv[:, 0], sc(2), None, Alu.mult)
                stt(t2[:], gv[:, 1], sc(5), t1[:], Alu.mult, Alu.add)
                stt(t3[:], gv[:, 2], sc(8), t2[:], Alu.mult, Alu.add)
                # fw = (gz<0) + 0.5*(gz==0)
                nc.vector.tensor_scalar(t1[:], t3[:], 0.0, None, Alu.is_lt)
                nc.vector.tensor_scalar(t2[:], t3[:], 0.0, None, Alu.is_equal)
                stt(fw_b[:, S], t2[:], 0.5, t1[:], Alu.mult, Alu.add)

                nc.vector.reciprocal(rr_b[:, S], zp[:])

                # pixel coords: clip(rint(coord/z), 0, 223) via +2^23
                def roundclamp(t):
                    stt(t1[:], t[:], 1.0, rr_b[:, S], Alu.mult, Alu.mult)
                    nc.vector.tensor_scalar(t2[:], t1[:], 224.0, MAGIC,
                                            Alu.min, Alu.add)
                    nc.vector.tensor_scalar(t[:], t2[:], MAGIC, 223.0,
                                            Alu.subtract, Alu.min)

                roundclamp(xp)
                roundclamp(yp)

                # int path: xi = int(xf); par = xi & 1; k = (xi>>1) + 112*yf
                xi = t1[:].bitcast(dt.int32)
                nc.vector.tensor_scalar(xi, xp[:], 0, None, Alu.bypass)
                pari = t3[:].bitcast(dt.int32)
                nc.vector.tensor_scalar(pari, xi, 1, None, Alu.bitwise_and)
                nc.vector.tensor_scalar(par_b[:, S], pari, 0, None, Alu.bypass)
                xh = t2[:].bitcast(dt.int32)
                nc.vector.tensor_scalar(xh, xi, 1, None, Alu.arith_shift_right)
                kf = kfpool.tile([128, 256], dt.float32, tag="kf")
                stt(kf[:], yp[:], 112.0, xh, Alu.mult, Alu.add)
                if DEBUG:
                    nc.sync.dma_start(dbg_kf.ap()[s], kf[:])

                # wrap k: 16 PE transposes -> PSUM[0:16, (u,p) @128u+p]
                ktr = pspool.tile([16, 2048], dt.float32, tag="ktr",
                                  name=f"ktr_{s}")
                for u in range(16):
                    nc.tensor.transpose(
                        ktr[:, 128 * u:128 * u + 128],
                        kf[:, 16 * u:16 * u + 16],
                        ident[:], tile_position=(0, 0))
                # stg16[32s+r, 1024h+q] = k of stream (h, t=16q+r)
                nc.scalar.copy(
                    stg16[32 * s:32 * s + 16, :],
                    ktr[:].rearrange("p (u h a) -> p h a u", u=16, h=2))
                for h in range(2):
                    nc.scalar.dma_start(
                        idxt[32 * s + 16 * h:32 * s + 16 * h + 16, :],
                        stg16[32 * s:32 * s + 16, 1024 * h:1024 * h + 1024])

                # d = base - gt ; lw = (gt<0.1)*fw
                stt(d_b[:, S], gtv[:], -1.0, bsv[:], Alu.mult, Alu.add)
                nc.vector.tensor_scalar(t2[:], gtv[:], 0.1, None, Alu.is_lt)
                stt(lw_b[:, S], t2[:], 1.0, fw_b[:, S], Alu.mult, Alu.mult)

                # ---------- images + stencils + bf16 map tiles ----------
                mapbf = {m: mbfpool.tile([112, 448], dt.bfloat16, tag=f"mb{m}",
                                         name=f"mapbf{m}_{s}", bufs=1)
                         for m in range(4)}

                imgF = load_img("imgF", pdp_d.ap()[s, 0], False)
                fshF = load_img("fshF", pdp_d.ap()[s, 0], True)
                nc.vector.tensor_scalar(
                    mapbf[0][:],
                    imgF[:].rearrange("p (r c) -> p r c", r=2)[:, :, 2:226],
                    1.0, None, Alu.mult)
                imgB = load_img("img5", pdp_d.ap()[s, 1], False)
                nc.vector.tensor_scalar(
                    mapbf[1][:],
                    imgB[:].rearrange("p (r c) -> p r c", r=2)[:, :, 2:226],
                    1.0, None, Alu.mult)

                fr = imgF[:].rearrange("p (r c) -> p r c", r=2)
                f_x0 = fr[:, :, 2:226]
                f_x1 = fr[:, :, 1:225]
                f_x2 = fr[:, :, 0:224]
                # nx2 = f - 2*f(x-1) + f(x-2)
                nx2 = stnpool.tile([112, 448], dt.float32, tag="nx2")
                nxv = nx2[:].rearrange("p (r c) -> p r c", r=2)
                stt(nxv, f_x1, -2.0, f_x0, Alu.mult, Alu.add)
                stt(nxv, f_x2, 1.0, nxv, Alu.mult, Alu.add)
                # ny2 = f - 2*f(y+1) + f(y+2) using the shifted copy fshF
                ny2 = stnpool.tile([112, 448], dt.float32, tag="ny2")
                f0, f1 = imgF[:, ROW0], imgF[:, ROW1]
                s0, s1 = fshF[:, ROW0], fshF[:, ROW1]
                stt(ny2[:, 0:224], f1, -2.0, f0, Alu.mult, Alu.add)
                stt(ny2[:, 0:224], s0, 1.0, ny2[:, 0:224], Alu.mult, Alu.add)
                stt(ny2[:, 224:448], s0, -2.0, f1, Alu.mult, Alu.add)
                stt(ny2[:, 224:448], s1, 1.0, ny2[:, 224:448], Alu.mult,
                    Alu.add)
                # P (unhalved) -> bf16 (interleave even/odd rows back)
                mb3 = mapbf[3][:].rearrange("p (r c) -> p r c", r=2)
                stt(mb3[:, 0], nxv[:, 0], 1.0, ny2[:, 0:224], Alu.mult,
                    Alu.add)
                stt(mb3[:, 1], nxv[:, 1], 1.0, ny2[:, 224:448], Alu.mult,
                    Alu.add)

                # G (unhalved): (mc5 - mc5(x-1)) + (mc4 - mc4(y+1))
                img4 = load_img("img4", mc_d.ap()[s, 0], False)
                fsh4 = load_img("fsh4", mc_d.ap()[s, 0], True)
                img5 = load_img("img5", mc_d.ap()[s, 1], False)
                g5 = img5[:].rearrange("p (r c) -> p r c", r=2)
                g1 = stnpool.tile([112, 448], dt.float32, tag="g1")
                g1v = g1[:].rearrange("p (r c) -> p r c", r=2)
                stt(g1v, g5[:, :, 1:225], -1.0, g5[:, :, 2:226], Alu.mult,
                    Alu.add)
                q0, q1 = img4[:, ROW0], img4[:, ROW1]
                w0 = fsh4[:, ROW0]
                ve = stnpool.tile([112, 448], dt.float32, tag="nx2",
                                  name=f"ve_{s}")
                stt(ve[:, 0:224], q1, -1.0, q0, Alu.mult, Alu.add)
                stt(ve[:, 224:448], w0, -1.0, q1, Alu.mult, Alu.add)
                mb2 = mapbf[2][:].rearrange("p (r c) -> p r c", r=2)
                stt(mb2[:, 0], g1v[:, 0], 1.0, ve[:, 0:224], Alu.mult,
                    Alu.add)
                stt(mb2[:, 1], g1v[:, 1], 1.0, ve[:, 224:448], Alu.mult,
                    Alu.add)

                # table DMAs to the h=0 group; one h=1 dup copy per sample
                for m in range(4):
                    p0 = 32 * s + m
                    eng = nc.sync if m % 2 == 0 else nc.scalar
                    eng.dma_start(
                        tabs[p0:p0 + 1, :].rearrange("p (a c) -> p a c",
                                                     a=112),
                        mapbf[m][:].unsqueeze(1))
                nc.scalar.dma_start(tabs[32 * s + 16:32 * s + 20, :],
                                    tabs[32 * s:32 * s + 4, :])

            if DEBUG:
                nc.sync.dma_start(dbg_idx.ap()[:], idxt[:])
                nc.sync.dma_start(dbg_tabs.ap()[:], tabs[:])

            # ---------- gathers + repacks ----------
            # pairs: col = m*2048 + s*512 + 2c + e  (payload col = 256s + c)
            pairs = bigpool.tile([128, 16 * 512], dt.bfloat16, tag="pairs")
            npart = NIC // 256
            for call in range(NCALL):
                gout = gopool.tile([128, 2 * NIC], dt.bfloat16, tag="gout",
                                   name=f"gout{call}")
                nc.gpsimd.ap_gather(
                    gout[:].rearrange("p (k e) -> p k e", e=2),
                    tabs[:].rearrange("p (k e) -> p k e", e=2),
                    idxt[:, 512 * call:512 * call + 512],
                    channels=128, num_elems=NE, d=2, num_idxs=NIC)
                for s in range(SPC):
                    for m in range(4):
                        blk = m * 2048 + s * 512
                        for h in range(2):
                            p0 = 64 * h + npart * call
                            eng = nc.scalar if (m + h) % 2 == 0 else nc.sync
                            eng.dma_start(
                                pairs[p0:p0 + npart, blk:blk + 512]
                                .unsqueeze(1),
                                gout[32 * s + 16 * h + m:
                                     32 * s + 16 * h + m + 1, :]
                                .rearrange("p (a c) -> p a c", a=npart))

            if DEBUG:
                nc.sync.dma_start(dbg_pairs.ap()[:], pairs[:])

            # ---------- select + losses (4-sample-wide) ----------
            sel = {}
            for m in range(4):
                pv2 = pairs[:, 2048 * m:2048 * m + 2048].rearrange(
                    "p (n e) -> p n e", e=2)
                nc.vector.copy_predicated(pv2[:, :, 0], par_b[:], pv2[:, :, 1])
                sel[m] = pv2[:, :, 0]

            b1 = tmppool.tile([128, 1024], dt.float32, tag="t1", name="b1")
            b2 = tmppool.tile([128, 1024], dt.float32, tag="t2", name="b2")

            # acc cols: 0 sq, 1 fw, 2 front, 3 back, 4 lw, 5 term
            stt(b1[:], d_b[:], 1.0, d_b[:], Alu.mult, Alu.mult,
                accum=acc[:, 0:1])
            nc.vector.tensor_scalar(b1[:], fw_b[:], 1.0, 0.0, Alu.mult,
                                    Alu.add, accum_out=acc[:, 1:2])
            nc.vector.tensor_scalar(b1[:], lw_b[:], 1.0, 0.0, Alu.mult,
                                    Alu.add, accum_out=acc[:, 4:5])

            stt(b1[:], d_b[:], 1.0, sel[0], Alu.mult, Alu.add)
            nc.scalar.activation(b1[:], b1[:], Act.Abs)
            stt(b2[:], b1[:], 1.0, fw_b[:], Alu.mult, Alu.mult,
                accum=acc[:, 2:3])

            nc.vector.tensor_scalar(b2[:], fw_b[:], -1.0, 1.0, Alu.mult,
                                    Alu.add)
            stt(b1[:], d_b[:], 1.0, sel[1], Alu.mult, Alu.add)
            nc.scalar.activation(b1[:], b1[:], Act.Abs)
            stt(b1[:], b1[:], 1.0, b2[:], Alu.mult, Alu.mult,
                accum=acc[:, 3:4])

            # laplacian: s' = gP*(24.5*scale)/z + gG (= 2x ref; host /4)
            stt(b2[:], sel[3], 1.0, rr_b[:], Alu.mult, Alu.mult)
            stt(b1[:], b2[:], scal[:, 49:50], sel[2], Alu.mult, Alu.add)
            nc.scalar.activation(b2[:], b1[:], Act.Square)
            stt(b1[:], b2[:], 1.0, lw_b[:], Alu.mult, Alu.mult,
                accum=acc[:, 5:6])

            nc.sync.dma_start(acc_d.ap()[:], acc[:])

    nc.compile()
    return nc


def _get_nc():
    if "nc" not in _cache:
        _cache["nc"] = build_nc()
    return _cache["nc"]


def kernel(gt_points, gt_values, gt_gradients, mc_image, gt_transmat, scale,
           base_values, pred_disp):
    nc = _get_nc()
    gt_points = _f32(gt_points)
    gt_values = _f32(gt_values)
    gt_gradients = _f32(gt_gradients)
    mc45 = _f32(mc_image[:, 4:6])
    gt_transmat = _f32(gt_transmat)
    scale = _f32(scale)
    base_values = _f32(base_values)
    pred_disp = _f32(pred_disp)

    in_maps = []
    for c in range(NCORES):
        sl = slice(SPC * c, SPC * (c + 1))
        tms = np.zeros((1, 49), np.float32)
        tms[0, :48] = gt_transmat[sl].reshape(-1)
        tms[0, 48] = scale[0]
        in_maps.append({
            "pts": _f32(gt_points[sl]),
            "grd": _f32(gt_gradients[sl]),
            "gtv": _f32(gt_values[sl, :, 0]),
            "bsv": _f32(base_values[sl, :, 0]),
            "pdp": _f32(pred_disp[sl]),
            "mc45": _f32(mc45[sl]),
            "tms": tms,
        })

    res = run_bass_kernel_spmd(nc, in_maps, core_ids=list(range(NCORES)))

    sq = fwsum = fa = ba = lwsum = tm = 0.0
    for c in range(NCORES):
        a = res.results[c]["acc"].astype(np.float64)
        sq += a[:, 0].sum()
        fwsum += a[:, 1].sum()
        fa += a[:, 2].sum()
        ba += a[:, 3].sum()
        lwsum += a[:, 4].sum()
        tm += a[:, 5].sum()

    tot = float(B * N)
    loss_base = sq / tot
    loss_front = fa / fwsum
    loss_back = ba / (tot - fwsum)
    loss_sdf = 0.5 * (loss_front + loss_back)
    loss_lap = tm / (4.0 * lwsum)
    return np.array([loss_base, loss_sdf, loss_lap], dtype=np.float32)


# revision 22
# speedup vs baseline: 1.5487x; 1.4505x over previous
"""Trainium2 Bass kernel for nn_D2IM_Net (D2IM losses).

Self-contained: takes FULL inputs as numpy arrays, shards batch (B=32) over
8 NeuronCores (4 samples each), runs one Bass/Tile kernel SPMD, and reduces
per-partition accumulator columns on the host (f64) into the 3 scalar losses.

Per-core pipeline (4 samples):
  - transmat/scale scalars broadcast to all partitions via a ones-matmul (PE)
  - point projection via fused scalar_tensor_tensor chains (DVE, 2x mode)
  - pixel coords: jax's astype(int32) rounds to nearest-even on this path,
    reproduced exactly by the f32 +2^23 magic add
  - 3x3 finite-difference stencils: x-shifts via free-dim APs; y-shifts via
    DMA-shifted image copies (engine APs need 32-aligned partition bases)
  - the 4 gather maps (pdf, pdb, gtlap G, predlap P) stored as bf16
    full-image tables, one partition per (sample, idx-half, map); gathered
    with gpsimd ap_gather (d=2 pairs, k = idx>>1), parity-selected afterwards
  - k-index streams wrapped into ap_gather's mod-16 partition layout with
    16 PE transposes per sample into PSUM + one ACT f32->int16 convert +
    2 small fixup DMAs
  - per-point payloads (d, fw, lw, par, 1/z) live in 4-sample-wide
    [128, 1024] tiles so the post-gather select + loss phase runs as a
    handful of wide fused ops with accum_out columns
  - the [128, 8] accumulator tile is DMAd out and reduced on the host (f64)
"""

import os
import sys

import numpy as np

for _p in ("/opt/trn_rl_repo", "/root/.axon_site/_ro/trn_rl_repo"):
    if os.path.isdir(_p) and _p not in sys.path:
        sys.path.insert(0, _p)

import concourse.bacc as bacc
import concourse.mybir as mybir
import concourse.tile as tile
from concourse.bass_utils import run_bass_kernel_spmd

dt = mybir.dt
Alu = mybir.AluOpType
Act = mybir.ActivationFunctionType

B, N, RES = 32, 32768, 224
NCORES = 8
SPC = B // NCORES            # samples per core = 4
NPIX = RES * RES             # 50176
NE = NPIX // 2               # 25088 bf16 pairs per table
NI = N // 2                  # 16384 idxs per (sample, half) group
NCALL = 1                    # gather split into NCALL calls
NIC = NI // NCALL            # idxs per call per group
MAGIC = 8388608.0            # 2^23
DEBUG = os.environ.get("D2IM_DEBUG") == "1"

_cache = {}


def _f32(x):
    return np.ascontiguousarray(x, dtype=np.float32)


def build_nc():
    nc = bacc.Bacc("TRN2", target_bir_lowering=False, debug=False,
                   enable_asserts=False, num_devices=NCORES)

    pts_d = nc.dram_tensor("pts", [SPC, N, 3], dt.float32, kind="ExternalInput")
    grd_d = nc.dram_tensor("grd", [SPC, N, 3], dt.float32, kind="ExternalInput")
    gtv_d = nc.dram_tensor("gtv", [SPC, N], dt.float32, kind="ExternalInput")
    bsv_d = nc.dram_tensor("bsv", [SPC, N], dt.float32, kind="ExternalInput")
    pdp_d = nc.dram_tensor("pdp", [SPC, 2, RES, RES], dt.float32,
                           kind="ExternalInput")
    mc_d = nc.dram_tensor("mc45", [SPC, 2, RES, RES], dt.float32,
                          kind="ExternalInput")
    tms_d = nc.dram_tensor("tms", [1, 49], dt.float32, kind="ExternalInput")
    acc_d = nc.dram_tensor("acc", [128, 8], dt.float32, kind="ExternalOutput")
    if DEBUG:
        dbg_kf = nc.dram_tensor("dbg_kf", [SPC, 128, 256], dt.float32,
                                kind="ExternalOutput")
        dbg_idx = nc.dram_tensor("dbg_idx", [128, 1024], dt.int16,
                                 kind="ExternalOutput")
        dbg_pairs = nc.dram_tensor("dbg_pairs", [128, 8192], dt.bfloat16,
                                   kind="ExternalOutput")
        dbg_tabs = nc.dram_tensor("dbg_tabs", [128, NPIX], dt.bfloat16,
                                  kind="ExternalOutput")

    from contextlib import ExitStack
    with tile.TileContext(nc) as tc:
        with tc.tile_pool(name="const", bufs=1) as cpool, \
             tc.tile_pool(name="payp", bufs=1) as paypool, \
             tc.tile_pool(name="bigp", bufs=1) as bigpool, \
             tc.tile_pool(name="psp", bufs=1, space="PSUM") as pspool:
            pre = ExitStack()
            ptspool = pre.enter_context(tc.tile_pool(name="ptsp", bufs=1))
            valpool = pre.enter_context(tc.tile_pool(name="valp", bufs=1))
            imgpool = pre.enter_context(tc.tile_pool(name="imgp", bufs=1))
            stnpool = pre.enter_context(tc.tile_pool(name="stnp", bufs=1))
            mbfpool = pre.enter_context(tc.tile_pool(name="mbfp", bufs=1))
            kfpool = pre.enter_context(tc.tile_pool(name="kfp", bufs=1))
            tmppool = pre.enter_context(tc.tile_pool(name="tmpp", bufs=1))

            # ---------- constants ----------
            ident = cpool.tile([128, 128], dt.float32, tag="ident")
            pidx = cpool.tile([128, 1], dt.int32, tag="pidx")
            icol_t = tmppool.tile([128, 256], dt.float32, tag="t1",
                                  name="icol_t")
            icol = icol_t[:, 0:128].bitcast(dt.int32)
            nc.gpsimd.iota(icol, [[1, 128]], base=0, channel_multiplier=0)
            nc.gpsimd.iota(pidx[:], [[0, 1]], base=0, channel_multiplier=1)
            nc.vector.tensor_tensor(ident[:], icol,
                                    pidx[:].broadcast_to((128, 128)),
                                    Alu.is_equal)

            stage = cpool.tile([1, 49], dt.float32, tag="stage")
            nc.sync.dma_start(stage[:], tms_d.ap()[:])
            ones1 = cpool.tile([1, 128], dt.float32, tag="ones1")
            nc.vector.memset(ones1[:], 1.0)
            bc_ps = pspool.tile([128, 64], dt.float32, tag="bcps")
            nc.tensor.matmul(bc_ps[:, 0:49], ones1[:], stage[:],
                             start=True, stop=True)
            scal = cpool.tile([128, 64], dt.float32, tag="scal")
            nc.scalar.copy(scal[:, 0:49], bc_ps[:, 0:49])
            # col 49: c2 = 2*49*scale/4 = 24.5*scale  (host divides sq-sum by 4)
            nc.vector.tensor_scalar(scal[:, 49:50], scal[:, 48:49], 24.5, None,
                                    Alu.mult)

            acc = cpool.tile([128, 8], dt.float32, tag="acc")
            nc.vector.memset(acc[:], 0.0)

            # gather tables: one partition per (s, h, m): partition 32s+16h+m
            tabs = bigpool.tile([128, NPIX], dt.bfloat16, tag="tabs")
            # wrapped int16 k-indices: partition 32s+16h+r, slot q
            idxt = cpool.tile([128, NI // 16], dt.int16, tag="idxt")
            # int16 staging before the 16-partition-offset fixup DMA
            stg16 = cpool.tile([128, 2048], dt.int16, tag="stg16")

            # 4-sample-wide payload tiles; col = 256*s + c for point (p, c)
            d_b = paypool.tile([128, 1024], dt.float32, tag="d")
            rr_b = paypool.tile([128, 1024], dt.float32, tag="rr")
            fw_b = paypool.tile([128, 1024], dt.bfloat16, tag="fw")
            lw_b = paypool.tile([128, 1024], dt.bfloat16, tag="lw")
            par_b = paypool.tile([128, 1024], dt.uint8, tag="par")

            ROW0 = slice(2, 226)     # data cols of row r=0 in padded imgs
            ROW1 = slice(228, 452)

            def load_img(name, dram_ap, shifted):
                """[112, 452] tile; partition p = rows (2p, 2p+1), each row at
                2 + 226*r with 2 zero pad cols. shifted=True: rows (2p+2,
                2p+3) for the y+1/y+2 taps; rows >= 224 are zeros."""
                t = imgpool.tile([112, 452], dt.float32, tag=name, name=name)
                if shifted:
                    nc.gpsimd.memset(t[96:112, :], 0.0)
                    nc.gpsimd.memset(
                        t[:].rearrange("p (r c) -> p r c", r=2)[:, :, 0:2],
                        0.0)
                    nc.sync.dma_start(
                        t[0:111, :].rearrange("p (r c) -> p r c", r=2)
                        [:, :, 2:226],
                        dram_ap[2:224].rearrange("(p r) c -> p r c", r=2))
                else:
                    nc.gpsimd.memset(
                        t[:].rearrange("p (r c) -> p r c", r=2)[:, :, 0:2],
                        0.0)
                    nc.sync.dma_start(
                        t[:].rearrange("p (r c) -> p r c", r=2)[:, :, 2:226],
                        dram_ap.rearrange("(p r) c -> p r c", r=2))
                return t

            def stt(out, in0, scalar, in1, op0, op1, accum=None):
                nc.vector.scalar_tensor_tensor(out, in0, scalar, in1, op0,
                                               op1, accum_out=accum)

            # ---------- batched per-point input loads ----------
            pts_b = ptspool.tile([128, 3072], dt.float32, tag="pts")
            nc.sync.dma_start(
                pts_b[:].rearrange("p (s n c) -> p s n c", s=SPC, c=3),
                pts_d.ap().rearrange("s (p n) c -> p s n c", p=128))
            grd_b = ptspool.tile([128, 3072], dt.float32, tag="grd")
            nc.sync.dma_start(
                grd_b[:].rearrange("p (s n c) -> p s n c", s=SPC, c=3),
                grd_d.ap().rearrange("s (p n) c -> p s n c", p=128))
            gtv_b = valpool.tile([128, 1024], dt.float32, tag="gtv")
            nc.sync.dma_start(
                gtv_b[:].rearrange("p (s n) -> p s n", s=SPC),
                gtv_d.ap().rearrange("s (p n) -> p s n", p=128))
            bsv_b = valpool.tile([128, 1024], dt.float32, tag="bsv")
            nc.sync.dma_start(
                bsv_b[:].rearrange("p (s n) -> p s n", s=SPC),
                bsv_d.ap().rearrange("s (p n) -> p s n", p=128))

            # ---------- per-sample pre-gather ----------
            for s in range(SPC):
                S = slice(256 * s, 256 * s + 256)
                gtv = gtv_b[:, S]
                bsv = bsv_b[:, S]
                pv = pts_b[:, 768 * s:768 * s + 768].rearrange(
                    "p (n c) -> p c n", c=3)
                gv = grd_b[:, 768 * s:768 * s + 768].rearrange(
                    "p (n c) -> p c n", c=3)

                def sc(f):
                    return scal[:, 12 * s + f:12 * s + f + 1]

                def tmp(tag):
                    return tmppool.tile([128, 256], dt.float32, tag=tag,
                                        name=f"{tag}_{s}")

                t1, t2, t3 = tmp("t1"), tmp("t2"), tmp("t3")
                xp, yp, zp = tmp("xp"), tmp("yp"), tmp("zp")

                # projection: col j: sum_i coord_i*T[i][j] + T[3][j]
                for (dst, j) in ((xp, 0), (yp, 1), (zp, 2)):
                    nc.vector.tensor_scalar(t1[:], pv[:, 0], sc(0 + j),
                                            sc(9 + j), Alu.mult, Alu.add)
                    stt(t2[:], pv[:, 1], sc(3 + j), t1[:], Alu.mult, Alu.add)
                    stt(dst[:], pv[:, 2], sc(6 + j), t2[:], Alu.mult, Alu.add)
                # grad z-projection (homogeneous pad is 0)
                nc.vector.tensor_scalar(t1[:], gv[:, 0], sc(2), None, Alu.mult)
                stt(t2[:], gv[:, 1], sc(5), t1[:], Alu.mult, Alu.add)
                stt(t3[:], gv[:, 2], sc(8), t2[:], Alu.mult, Alu.add)
                # fw = (gz<0) + 0.5*(gz==0)
                nc.vector.tensor_scalar(t1[:], t3[:], 0.0, None, Alu.is_lt)
                nc.vector.tensor_scalar(t2[:], t3[:], 0.0, None, Alu.is_equal)
                stt(fw_b[:, S], t2[:], 0.5, t1[:], Alu.mult, Alu.add)

                nc.vector.reciprocal(rr_b[:, S], zp[:])

                # pixel coords: clip(rint(coord/z), 0, 223) via +2^23
                def roundclamp(t):
                    stt(t1[:], t[:], 1.0, rr_b[:, S], Alu.mult, Alu.mult)
                    nc.vector.tensor_scalar(t2[:], t1[:], 224.0, MAGIC,
                                            Alu.min, Alu.add)
                    nc.vector.tensor_scalar(t[:], t2[:], MAGIC, 223.0,
                                            Alu.subtract, Alu.min)

                roundclamp(xp)
                roundclamp(yp)

                # int path: xi = int(xf); par = xi & 1; k = (xi>>1) + 112*yf
                xi = t1[:].bitcast(dt.int32)
                nc.vector.tensor_scalar(xi, xp[:], 0, None, Alu.bypass)
                pari = t3[:].bitcast(dt.int32)
                nc.vector.tensor_scalar(pari, xi, 1, None, Alu.bitwise_and)
                nc.vector.tensor_scalar(par_b[:, S], pari, 0, None, Alu.bypass)
                xh = t2[:].bitcast(dt.int32)
                nc.vector.tensor_scalar(xh, xi, 1, None, Alu.arith_shift_right)
                kf = kfpool.tile([128, 256], dt.float32, tag="kf")
                stt(kf[:], yp[:], 112.0, xh, Alu.mult, Alu.add)
                if DEBUG:
                    nc.sync.dma_start(dbg_kf.ap()[s], kf[:])

                # wrap k: 16 PE transposes -> PSUM[0:16, (u,p) @128u+p]
                ktr = pspool.tile([16, 2048], dt.float32, tag="ktr",
                                  name=f"ktr_{s}")
                for u in range(16):
                    nc.tensor.transpose(
                        ktr[:, 128 * u:128 * u + 128],
                        kf[:, 16 * u:16 * u + 16],
                        ident[:], tile_position=(0, 0))
                # stg16[32s+r, 1024h+q] = k of stream (h, t=16q+r)
                nc.scalar.copy(
                    stg16[32 * s:32 * s + 16, :],
                    ktr[:].rearrange("p (u h a) -> p h a u", u=16, h=2))
                for h in range(2):
                    nc.scalar.dma_start(
                        idxt[32 * s + 16 * h:32 * s + 16 * h + 16, :],
                        stg16[32 * s:32 * s + 16, 1024 * h:1024 * h + 1024])

                # d = base - gt ; lw = (gt<0.1)*fw
                stt(d_b[:, S], gtv, -1.0, bsv, Alu.mult, Alu.add)
                nc.vector.tensor_scalar(t2[:], gtv, 0.1, None, Alu.is_lt)
                stt(lw_b[:, S], t2[:], 1.0, fw_b[:, S], Alu.mult, Alu.mult)

                # ---------- images + stencils + bf16 map tiles ----------
                mapbf = {m: mbfpool.tile([112, 448], dt.bfloat16, tag=f"mb{m}",
                                         name=f"mapbf{m}_{s}", bufs=1)
                         for m in range(4)}

                imgF = load_img("imgF", pdp_d.ap()[s, 0], False)
                fshF = load_img("fshF", pdp_d.ap()[s, 0], True)
                nc.vector.tensor_scalar(
                    mapbf[0][:],
                    imgF[:].rearrange("p (r c) -> p r c", r=2)[:, :, 2:226],
                    1.0, None, Alu.mult)
                imgB = load_img("img5", pdp_d.ap()[s, 1], False)
                nc.vector.tensor_scalar(
                    mapbf[1][:],
                    imgB[:].rearrange("p (r c) -> p r c", r=2)[:, :, 2:226],
                    1.0, None, Alu.mult)

                fr = imgF[:].rearrange("p (r c) -> p r c", r=2)
                f_x0 = fr[:, :, 2:226]
                f_x1 = fr[:, :, 1:225]
                f_x2 = fr[:, :, 0:224]
                # nx2 = f - 2*f(x-1) + f(x-2)
                nx2 = stnpool.tile([112, 448], dt.float32, tag="nx2")
                nxv = nx2[:].rearrange("p (r c) -> p r c", r=2)
                stt(nxv, f_x1, -2.0, f_x0, Alu.mult, Alu.add)
                stt(nxv, f_x2, 1.0, nxv, Alu.mult, Alu.add)
                # ny2 = f - 2*f(y+1) + f(y+2) using the shifted copy fshF
                ny2 = stnpool.tile([112, 448], dt.float32, tag="ny2")
                f0, f1 = imgF[:, ROW0], imgF[:, ROW1]
                s0, s1 = fshF[:, ROW0], fshF[:, ROW1]
                stt(ny2[:, 0:224], f1, -2.0, f0, Alu.mult, Alu.add)
                stt(ny2[:, 0:224], s0, 1.0, ny2[:, 0:224], Alu.mult, Alu.add)
                stt(ny2[:, 224:448], s0, -2.0, f1, Alu.mult, Alu.add)
                stt(ny2[:, 224:448], s1, 1.0, ny2[:, 224:448], Alu.mult,
                    Alu.add)
                # P (unhalved) -> bf16 (interleave even/odd rows back)
                mb3 = mapbf[3][:].rearrange("p (r c) -> p r c", r=2)
                stt(mb3[:, 0], nxv[:, 0], 1.0, ny2[:, 0:224], Alu.mult,
                    Alu.add)
                stt(mb3[:, 1], nxv[:, 1], 1.0, ny2[:, 224:448], Alu.mult,
                    Alu.add)

                # G (unhalved): (mc5 - mc5(x-1)) + (mc4 - mc4(y+1))
                img4 = load_img("img4", mc_d.ap()[s, 0], False)
                fsh4 = load_img("fsh4", mc_d.ap()[s, 0], True)
                img5 = load_img("img5", mc_d.ap()[s, 1], False)
                g5 = img5[:].rearrange("p (r c) -> p r c", r=2)
                g1 = stnpool.tile([112, 448], dt.float32, tag="g1")
                g1v = g1[:].rearrange("p (r c) -> p r c", r=2)
                stt(g1v, g5[:, :, 1:225], -1.0, g5[:, :, 2:226], Alu.mult,
                    Alu.add)
                q0, q1 = img4[:, ROW0], img4[:, ROW1]
                w0 = fsh4[:, ROW0]
                ve = stnpool.tile([112, 448], dt.float32, tag="nx2",
                                  name=f"ve_{s}")
                stt(ve[:, 0:224], q1, -1.0, q0, Alu.mult, Alu.add)
                stt(ve[:, 224:448], w0, -1.0, q1, Alu.mult, Alu.add)
                mb2 = mapbf[2][:].rearrange("p (r c) -> p r c", r=2)
                stt(mb2[:, 0], g1v[:, 0], 1.0, ve[:, 0:224], Alu.mult,
                    Alu.add)
                stt(mb2[:, 1], g1v[:, 1], 1.0, ve[:, 224:448], Alu.mult,
                    Alu.add)

                # table DMAs to the h=0 group; one h=1 dup copy per sample
                for m in range(4):
                    p0 = 32 * s + m
                    eng = nc.sync if m % 2 == 0 else nc.scalar
                    eng.dma_start(
                        tabs[p0:p0 + 1, :].rearrange("p (a c) -> p a c",
                                                     a=112),
                        mapbf[m][:].unsqueeze(1))
                nc.scalar.dma_start(tabs[32 * s + 16:32 * s + 20, :],
                                    tabs[32 * s:32 * s + 4, :])

            if DEBUG:
                nc.sync.dma_start(dbg_idx.ap()[:], idxt[:])
                nc.sync.dma_start(dbg_tabs.ap()[:], tabs[:])

            pre.close()
            post = ExitStack()
            gopool = post.enter_context(tc.tile_pool(name="gop", bufs=1))

            # ---------- gathers + repacks ----------
            # pairs: col = m*2048 + s*512 + 2c + e  (payload col = 256s + c)
            pairs = bigpool.tile([128, 16 * 512], dt.bfloat16, tag="pairs")
            npart = NIC // 256
            for call in range(NCALL):
                gout = gopool.tile([128, 2 * NIC], dt.bfloat16, tag="gout",
                                   name=f"gout{call}")
                nc.gpsimd.ap_gather(
                    gout[:].rearrange("p (k e) -> p k e", e=2),
                    tabs[:].rearrange("p (k e) -> p k e", e=2),
                    idxt[:, (NIC // 16) * call:(NIC // 16) * (call + 1)],
                    channels=128, num_elems=NE, d=2, num_idxs=NIC)
                for s in range(SPC):
                    for m in range(4):
                        blk = m * 2048 + s * 512
                        for h in range(2):
                            p0 = 64 * h + npart * call
                            eng = nc.scalar if (m + h) % 2 == 0 else nc.sync
                            eng.dma_start(
                                pairs[p0:p0 + npart, blk:blk + 512]
                                .unsqueeze(1),
                                gout[32 * s + 16 * h + m:
                                     32 * s + 16 * h + m + 1, :]
                                .rearrange("p (a c) -> p a c", a=npart))

            if DEBUG:
                nc.sync.dma_start(dbg_pairs.ap()[:], pairs[:])

            # ---------- select + losses (4-sample-wide) ----------
            sel = {}
            for m in range(4):
                pv2 = pairs[:, 2048 * m:2048 * m + 2048].rearrange(
                    "p (n e) -> p n e", e=2)
                nc.vector.copy_predicated(pv2[:, :, 0], par_b[:], pv2[:, :, 1])
                sel[m] = pv2[:, :, 0]

            b1 = gopool.tile([128, 1024], dt.float32, tag="b1", name="b1")
            b2 = gopool.tile([128, 1024], dt.float32, tag="b2", name="b2")

            # acc cols: 0 sq, 1 fw, 2 front, 3 back, 4 lw, 5 term
            stt(b1[:], d_b[:], 1.0, d_b[:], Alu.mult, Alu.mult,
                accum=acc[:, 0:1])
            nc.vector.tensor_scalar(b1[:], fw_b[:], 1.0, 0.0, Alu.mult,
                                    Alu.add, accum_out=acc[:, 1:2])
            nc.vector.tensor_scalar(b1[:], lw_b[:], 1.0, 0.0, Alu.mult,
                                    Alu.add, accum_out=acc[:, 4:5])

            stt(b1[:], d_b[:], 1.0, sel[0], Alu.mult, Alu.add)
            nc.scalar.activation(b1[:], b1[:], Act.Abs)
            stt(b2[:], b1[:], 1.0, fw_b[:], Alu.mult, Alu.mult,
                accum=acc[:, 2:3])

            nc.vector.tensor_scalar(b2[:], fw_b[:], -1.0, 1.0, Alu.mult,
                                    Alu.add)
            stt(b1[:], d_b[:], 1.0, sel[1], Alu.mult, Alu.add)
            nc.scalar.activation(b1[:], b1[:], Act.Abs)
            stt(b1[:], b1[:], 1.0, b2[:], Alu.mult, Alu.mult,
                accum=acc[:, 3:4])

            # laplacian: s' = gP*(24.5*scale)/z + gG (= 2x ref; host /4)
            stt(b2[:], sel[3], 1.0, rr_b[:], Alu.mult, Alu.mult)
            stt(b1[:], b2[:], scal[:, 49:50], sel[2], Alu.mult, Alu.add)
            nc.scalar.activation(b2[:], b1[:], Act.Square)
            stt(b1[:], b2[:], 1.0, lw_b[:], Alu.mult, Alu.mult,
                accum=acc[:, 5:6])

            nc.sync.dma_start(acc_d.ap()[:], acc[:])
            post.close()

    nc.compile()
    return nc


def _get_nc():
    if "nc" not in _cache:
        _cache["nc"] = build_nc()
    return _cache["nc"]


def kernel(gt_points, gt_values, gt_gradients, mc_image, gt_transmat, scale,
           base_values, pred_disp):
    nc = _get_nc()
    gt_points = _f32(gt_points)
    gt_values = _f32(gt_values)
    gt_gradients = _f32(gt_gradients)
    mc45 = _f32(mc_image[:, 4:6])
    gt_transmat = _f32(gt_transmat)
    scale = _f32(scale)
    base_values = _f32(base_values)
    pred_disp = _f32(pred_disp)

    in_maps = []
    for c in range(NCORES):
        sl = slice(SPC * c, SPC * (c + 1))
        tms = np.zeros((1, 49), np.float32)
        tms[0, :48] = gt_transmat[sl].reshape(-1)
        tms[0, 48] = scale[0]
        in_maps.append({
            "pts": _f32(gt_points[sl]),
            "grd": _f32(gt_gradients[sl]),
            "gtv": _f32(gt_values[sl, :, 0]),
            "bsv": _f32(base_values[sl, :, 0]),
            "pdp": _f32(pred_disp[sl]),
            "mc45": _f32(mc45[sl]),
            "tms": tms,
        })

    res = run_bass_kernel_spmd(nc, in_maps, core_ids=list(range(NCORES)))

    sq = fwsum = fa = ba = lwsum = tm = 0.0
    for c in range(NCORES):
        a = res.results[c]["acc"].astype(np.float64)
        sq += a[:, 0].sum()
        fwsum += a[:, 1].sum()
        fa += a[:, 2].sum()
        ba += a[:, 3].sum()
        lwsum += a[:, 4].sum()
        tm += a[:, 5].sum()

    tot = float(B * N)
    loss_base = sq / tot
    loss_front = fa / fwsum
    loss_back = ba / (tot - fwsum)
    loss_sdf = 0.5 * (loss_front + loss_back)
    loss_lap = tm / (4.0 * lwsum)
    return np.array([loss_base, loss_sdf, loss_lap], dtype=np.float32)


# revision 27
# speedup vs baseline: 1.5828x; 1.0220x over previous
"""Trainium2 Bass kernel for nn_D2IM_Net (D2IM losses).

Self-contained: takes FULL inputs as numpy arrays, shards batch (B=32) over
8 NeuronCores (4 samples each), runs one Bass/Tile kernel SPMD, and reduces
per-partition accumulator columns on the host (f64) into the 3 scalar losses.

Per-core pipeline (4 samples):
  - transmat/scale scalars broadcast to all partitions via a ones-matmul (PE)
  - point projection via fused scalar_tensor_tensor chains (DVE, 2x mode)
  - pixel coords: jax's astype(int32) rounds to nearest-even on this path,
    reproduced exactly by the f32 +2^23 magic add
  - 3x3 finite-difference stencils: x-shifts via free-dim APs; y-shifts via
    DMA-shifted image copies (engine APs need 32-aligned partition bases)
  - the 4 gather maps (pdf, pdb, gtlap G, predlap P) stored as bf16
    full-image tables, one partition per (sample, idx-half, map); gathered
    with gpsimd ap_gather (d=2 pairs, k = idx>>1), parity-selected afterwards
  - k-index streams wrapped into ap_gather's mod-16 partition layout with
    16 PE transposes per sample into PSUM + one ACT f32->int16 convert +
    2 small fixup DMAs
  - per-point payloads (d, fw, lw, par, 1/z) live in 4-sample-wide
    [128, 1024] tiles so the post-gather select + loss phase runs as a
    handful of wide fused ops with accum_out columns
  - the [128, 8] accumulator tile is DMAd out and reduced on the host (f64)
"""

import os
import sys

import numpy as np

for _p in ("/opt/trn_rl_repo", "/root/.axon_site/_ro/trn_rl_repo"):
    if os.path.isdir(_p) and _p not in sys.path:
        sys.path.insert(0, _p)

import concourse.bacc as bacc
import concourse.mybir as mybir
import concourse.tile as tile
from concourse.bass_utils import run_bass_kernel_spmd

dt = mybir.dt
Alu = mybir.AluOpType
Act = mybir.ActivationFunctionType

B, N, RES = 32, 32768, 224
NCORES = 8
SPC = B // NCORES            # samples per core = 4
NPIX = RES * RES             # 50176
NE = NPIX // 2               # 25088 bf16 pairs per table
NI = N // 2                  # 16384 idxs per (sample, half) group
NCALL = 1                    # gather split into NCALL calls
NIC = NI // NCALL            # idxs per call per group
MAGIC = 8388608.0            # 2^23
DEBUG = os.environ.get("D2IM_DEBUG") == "1"

_cache = {}


def _f32(x):
    return np.ascontiguousarray(x, dtype=np.float32)


def build_nc():
    nc = bacc.Bacc("TRN2", target_bir_lowering=False, debug=False,
                   enable_asserts=False, num_devices=NCORES)

    pts_d = nc.dram_tensor("pts", [SPC, N, 3], dt.float32, kind="ExternalInput")
    grd_d = nc.dram_tensor("grd", [SPC, N, 3], dt.float32, kind="ExternalInput")
    gtv_d = nc.dram_tensor("gtv", [SPC, N], dt.float32, kind="ExternalInput")
    bsv_d = nc.dram_tensor("bsv", [SPC, N], dt.float32, kind="ExternalInput")
    pdp_d = nc.dram_tensor("pdp", [SPC, 2, RES, RES], dt.float32,
                           kind="ExternalInput")
    mc_d = nc.dram_tensor("mc45", [SPC, 2, RES, RES], dt.float32,
                          kind="ExternalInput")
    tms_d = nc.dram_tensor("tms", [1, 49], dt.float32, kind="ExternalInput")
    acc_d = nc.dram_tensor("acc", [128, 8], dt.float32, kind="ExternalOutput")
    if DEBUG:
        dbg_kf = nc.dram_tensor("dbg_kf", [SPC, 128, 256], dt.float32,
                                kind="ExternalOutput")
        dbg_idx = nc.dram_tensor("dbg_idx", [128, 1024], dt.int16,
                                 kind="ExternalOutput")
        dbg_pairs = nc.dram_tensor("dbg_pairs", [128, 8192], dt.bfloat16,
                                   kind="ExternalOutput")
        dbg_tabs = nc.dram_tensor("dbg_tabs", [128, NPIX], dt.bfloat16,
                                  kind="ExternalOutput")

    from contextlib import ExitStack
    with tile.TileContext(nc) as tc:
        with tc.tile_pool(name="const", bufs=1) as cpool, \
             tc.tile_pool(name="payp", bufs=1) as paypool, \
             tc.tile_pool(name="bigp", bufs=1) as bigpool, \
             tc.tile_pool(name="psp", bufs=1, space="PSUM") as pspool:
            pre = ExitStack()
            ptspool = pre.enter_context(tc.tile_pool(name="ptsp", bufs=1))
            valpool = pre.enter_context(tc.tile_pool(name="valp", bufs=1))
            imgpool = pre.enter_context(tc.tile_pool(name="imgp", bufs=1))
            stnpool = pre.enter_context(tc.tile_pool(name="stnp", bufs=1))
            mbfpool = pre.enter_context(tc.tile_pool(name="mbfp", bufs=1))
            kfpool = pre.enter_context(tc.tile_pool(name="kfp", bufs=1))
            tmppool = pre.enter_context(tc.tile_pool(name="tmpp", bufs=1))

            # ---------- constants ----------
            ident = cpool.tile([128, 128], dt.float32, tag="ident")
            pidx = cpool.tile([128, 1], dt.int32, tag="pidx")
            icol_t = tmppool.tile([128, 256], dt.float32, tag="t1",
                                  name="icol_t")
            icol = icol_t[:, 0:128].bitcast(dt.int32)
            nc.gpsimd.iota(icol, [[1, 128]], base=0, channel_multiplier=0)
            nc.gpsimd.iota(pidx[:], [[0, 1]], base=0, channel_multiplier=1)
            nc.vector.tensor_tensor(ident[:], icol,
                                    pidx[:].broadcast_to((128, 128)),
                                    Alu.is_equal)

            stage = cpool.tile([1, 49], dt.float32, tag="stage")
            nc.sync.dma_start(stage[:], tms_d.ap()[:])
            ones1 = cpool.tile([1, 128], dt.float32, tag="ones1")
            nc.vector.memset(ones1[:], 1.0)
            bc_ps = pspool.tile([128, 64], dt.float32, tag="bcps")
            nc.tensor.matmul(bc_ps[:, 0:49], ones1[:], stage[:],
                             start=True, stop=True)
            scal = cpool.tile([128, 64], dt.float32, tag="scal")
            nc.scalar.copy(scal[:, 0:49], bc_ps[:, 0:49])
            # col 49: c2 = 2*49*scale/4 = 24.5*scale  (host divides sq-sum by 4)
            nc.vector.tensor_scalar(scal[:, 49:50], scal[:, 48:49], 24.5, None,
                                    Alu.mult)

            acc = cpool.tile([128, 8], dt.float32, tag="acc")
            nc.vector.memset(acc[:], 0.0)

            # gather tables: one partition per (s, h, m): partition 32s+16h+m
            tabs = bigpool.tile([128, NPIX], dt.bfloat16, tag="tabs")
            # wrapped int16 k-indices: partition 32s+16h+r, slot q
            idxt = cpool.tile([128, NI // 16], dt.int16, tag="idxt")
            # int16 staging before the 16-partition-offset fixup DMA
            stg16 = cpool.tile([128, 2048], dt.int16, tag="stg16")

            # 4-sample-wide payload tiles; col = 256*s + c for point (p, c)
            d_b = paypool.tile([128, 1024], dt.float32, tag="d")
            rr_b = paypool.tile([128, 1024], dt.float32, tag="rr")
            fw_b = paypool.tile([128, 1024], dt.bfloat16, tag="fw")
            lw_b = paypool.tile([128, 1024], dt.bfloat16, tag="lw")
            par_b = paypool.tile([128, 1024], dt.uint8, tag="par")

            ROW0 = slice(0, 224)     # row r=0 cols in unpadded image views
            ROW1 = slice(224, 448)

            def load_img2(name, dram2_ap, shifted, sp):
                """Two samples per unpadded [112, 896] tile: col =
                448*(s%2) + 224*r + x; partition p holds rows (2p, 2p+1).
                shifted=True: rows (2p+2, 2p+3); rows >= 224 zero."""
                t = imgpool.tile([112, 896], dt.float32, tag=name,
                                 name=f"{name}_{sp}")
                if shifted:
                    nc.gpsimd.memset(t[96:112, :], 0.0)
                    nc.sync.dma_start(
                        t[0:111, :].rearrange("p (s rx) -> p s rx", s=2),
                        dram2_ap[:, 2:224].rearrange(
                            "s (p r) x -> p s (r x)", r=2))
                else:
                    nc.sync.dma_start(
                        t[:].rearrange("p (s rx) -> p s rx", s=2),
                        dram2_ap.rearrange("s (p r) x -> p s (r x)", r=2))
                return t

            def stt(out, in0, scalar, in1, op0, op1, accum=None):
                nc.vector.scalar_tensor_tensor(out, in0, scalar, in1, op0,
                                               op1, accum_out=accum)

            # ---------- batched per-point input loads ----------
            pts_b = ptspool.tile([128, 3072], dt.float32, tag="pts")
            nc.sync.dma_start(
                pts_b[:].rearrange("p (s n c) -> p s n c", s=SPC, c=3),
                pts_d.ap().rearrange("s (p n) c -> p s n c", p=128))
            grd_b = ptspool.tile([128, 3072], dt.float32, tag="grd")
            nc.sync.dma_start(
                grd_b[:].rearrange("p (s n c) -> p s n c", s=SPC, c=3),
                grd_d.ap().rearrange("s (p n) c -> p s n c", p=128))
            gtv_b = valpool.tile([128, 1024], dt.float32, tag="gtv")
            nc.sync.dma_start(
                gtv_b[:].rearrange("p (s n) -> p s n", s=SPC),
                gtv_d.ap().rearrange("s (p n) -> p s n", p=128))
            bsv_b = valpool.tile([128, 1024], dt.float32, tag="bsv")
            nc.sync.dma_start(
                bsv_b[:].rearrange("p (s n) -> p s n", s=SPC),
                bsv_d.ap().rearrange("s (p n) -> p s n", p=128))

            # ---------- per-sample pre-gather ----------
            imgtiles = {}
            for s in range(SPC):
                S = slice(256 * s, 256 * s + 256)
                gtv = gtv_b[:, S]
                bsv = bsv_b[:, S]
                pv = pts_b[:, 768 * s:768 * s + 768].rearrange(
                    "p (n c) -> p c n", c=3)
                gv = grd_b[:, 768 * s:768 * s + 768].rearrange(
                    "p (n c) -> p c n", c=3)

                def sc(f):
                    return scal[:, 12 * s + f:12 * s + f + 1]

                def tmp(tag):
                    return tmppool.tile([128, 256], dt.float32, tag=tag,
                                        name=f"{tag}_{s}")

                t1, t2, t3 = tmp("t1"), tmp("t2"), tmp("t3")
                xp, yp, zp = tmp("xp"), tmp("yp"), tmp("zp")

                # projection: col j: sum_i coord_i*T[i][j] + T[3][j]
                for (dst, j) in ((xp, 0), (yp, 1), (zp, 2)):
                    nc.vector.tensor_scalar(t1[:], pv[:, 0], sc(0 + j),
                                            sc(9 + j), Alu.mult, Alu.add)
                    stt(t2[:], pv[:, 1], sc(3 + j), t1[:], Alu.mult, Alu.add)
                    stt(dst[:], pv[:, 2], sc(6 + j), t2[:], Alu.mult, Alu.add)
                # grad z-projection (homogeneous pad is 0)
                nc.vector.tensor_scalar(t1[:], gv[:, 0], sc(2), None, Alu.mult)
                stt(t2[:], gv[:, 1], sc(5), t1[:], Alu.mult, Alu.add)
                stt(t3[:], gv[:, 2], sc(8), t2[:], Alu.mult, Alu.add)
                # fw = (gz<0) + 0.5*(gz==0)
                nc.vector.tensor_scalar(t1[:], t3[:], 0.0, None, Alu.is_lt)
                nc.vector.tensor_scalar(t2[:], t3[:], 0.0, None, Alu.is_equal)
                stt(fw_b[:, S], t2[:], 0.5, t1[:], Alu.mult, Alu.add)

                nc.vector.reciprocal(rr_b[:, S], zp[:])

                # pixel coords: clip(rint(coord/z), 0, 223) via +2^23
                def roundclamp(t):
                    stt(t1[:], t[:], 1.0, rr_b[:, S], Alu.mult, Alu.mult)
                    nc.vector.tensor_scalar(t2[:], t1[:], 224.0, MAGIC,
                                            Alu.min, Alu.add)
                    nc.vector.tensor_scalar(t[:], t2[:], MAGIC, 223.0,
                                            Alu.subtract, Alu.min)

                roundclamp(xp)
                roundclamp(yp)

                # int path: xi = int(xf); par = xi & 1; k = (xi>>1) + 112*yf
                xi = t1[:].bitcast(dt.int32)
                nc.vector.tensor_scalar(xi, xp[:], 0, None, Alu.bypass)
                pari = t3[:].bitcast(dt.int32)
                nc.vector.tensor_scalar(pari, xi, 1, None, Alu.bitwise_and)
                nc.vector.tensor_scalar(par_b[:, S], pari, 0, None, Alu.bypass)
                xh = t2[:].bitcast(dt.int32)
                nc.vector.tensor_scalar(xh, xi, 1, None, Alu.arith_shift_right)
                kf = kfpool.tile([128, 256], dt.float32, tag="kf")
                stt(kf[:], yp[:], 112.0, xh, Alu.mult, Alu.add)
                if DEBUG:
                    nc.sync.dma_start(dbg_kf.ap()[s], kf[:])

                # wrap k: 16 PE transposes -> PSUM[0:16, (u,p) @128u+p]
                ktr = pspool.tile([16, 2048], dt.float32, tag="ktr",
                                  name=f"ktr_{s}")
                for u in range(16):
                    nc.tensor.transpose(
                        ktr[:, 128 * u:128 * u + 128],
                        kf[:, 16 * u:16 * u + 16],
                        ident[:], tile_position=(0, 0))
                # stg16[32s+r, 1024h+q] = k of stream (h, t=16q+r)
                nc.scalar.copy(
                    stg16[32 * s:32 * s + 16, :],
                    ktr[:].rearrange("p (u h a) -> p h a u", u=16, h=2))
                for h in range(2):
                    nc.scalar.dma_start(
                        idxt[32 * s + 16 * h:32 * s + 16 * h + 16, :],
                        stg16[32 * s:32 * s + 16, 1024 * h:1024 * h + 1024])

                # d = base - gt ; lw = (gt<0.1)*fw
                stt(d_b[:, S], gtv, -1.0, bsv, Alu.mult, Alu.add)
                nc.vector.tensor_scalar(t2[:], gtv, 0.1, None, Alu.is_lt)
                stt(lw_b[:, S], t2[:], 1.0, fw_b[:, S], Alu.mult, Alu.mult)

                # ---------- images + stencils + bf16 map tiles ----------
                mapbf = {m: mbfpool.tile([112, 448], dt.bfloat16, tag=f"mb{m}",
                                         name=f"mapbf{m}_{s}", bufs=1)
                         for m in range(4)}

                if s % 2 == 0:
                    sp = s
                    img2 = {
                        "imgF": load_img2("imgF", pdp_d.ap()[sp:sp + 2, 0],
                                          False, sp),
                        "fshF": load_img2("fshF", pdp_d.ap()[sp:sp + 2, 0],
                                          True, sp),
                        "imgB": load_img2("imgB", pdp_d.ap()[sp:sp + 2, 1],
                                          False, sp),
                        "img4": load_img2("img4", mc_d.ap()[sp:sp + 2, 0],
                                          False, sp),
                        "fsh4": load_img2("fsh4", mc_d.ap()[sp:sp + 2, 0],
                                          True, sp),
                        "img5": load_img2("img5", mc_d.ap()[sp:sp + 2, 1],
                                          False, sp),
                    }
                    imgtiles[0] = img2
                img2 = imgtiles[0]
                j = s % 2
                JS = slice(448 * j, 448 * j + 448)
                imgF = img2["imgF"][:, JS]
                fshF = img2["fshF"][:, JS]
                imgB = img2["imgB"][:, JS]
                img4v = img2["img4"][:, JS]
                fsh4v = img2["fsh4"][:, JS]
                img5v = img2["img5"][:, JS]
                nc.scalar.copy(mapbf[0][:], imgF)
                nc.scalar.copy(mapbf[1][:], imgB)

                fr = imgF.rearrange("p (r c) -> p r c", r=2)
                # nx2 = f - 2*f(x-1) + f(x-2)  (zero-padded at x<0)
                nx2 = stnpool.tile([112, 448], dt.float32, tag="nx2")
                nxv = nx2[:].rearrange("p (r c) -> p r c", r=2)
                stt(nxv[:, :, 2:224], fr[:, :, 1:223], -2.0, fr[:, :, 2:224],
                    Alu.mult, Alu.add)
                stt(nxv[:, :, 2:224], fr[:, :, 0:222], 1.0, nxv[:, :, 2:224],
                    Alu.mult, Alu.add)
                nc.vector.tensor_scalar(nxv[:, :, 0:1], fr[:, :, 0:1], 1.0,
                                        None, Alu.mult)
                stt(nxv[:, :, 1:2], fr[:, :, 0:1], -2.0, fr[:, :, 1:2],
                    Alu.mult, Alu.add)
                # ny2 = f - 2*f(y+1) + f(y+2) using the shifted copy fshF
                ny2 = stnpool.tile([112, 448], dt.float32, tag="ny2")
                f0, f1 = imgF[:, ROW0], imgF[:, ROW1]
                s0, s1 = fshF[:, ROW0], fshF[:, ROW1]
                stt(ny2[:, 0:224], f1, -2.0, f0, Alu.mult, Alu.add)
                stt(ny2[:, 0:224], s0, 1.0, ny2[:, 0:224], Alu.mult, Alu.add)
                stt(ny2[:, 224:448], s0, -2.0, f1, Alu.mult, Alu.add)
                stt(ny2[:, 224:448], s1, 1.0, ny2[:, 224:448], Alu.mult,
                    Alu.add)
                # P (unhalved) -> bf16 (interleave even/odd rows back)
                mb3 = mapbf[3][:].rearrange("p (r c) -> p r c", r=2)
                stt(mb3[:, 0], nxv[:, 0], 1.0, ny2[:, 0:224], Alu.mult,
                    Alu.add)
                stt(mb3[:, 1], nxv[:, 1], 1.0, ny2[:, 224:448], Alu.mult,
                    Alu.add)

                # G (unhalved): (mc5 - mc5(x-1)) + (mc4 - mc4(y+1))
                g5 = img5v.rearrange("p (r c) -> p r c", r=2)
                g1 = stnpool.tile([112, 448], dt.float32, tag="g1")
                g1v = g1[:].rearrange("p (r c) -> p r c", r=2)
                stt(g1v[:, :, 1:224], g5[:, :, 0:223], -1.0, g5[:, :, 1:224],
                    Alu.mult, Alu.add)
                nc.vector.tensor_scalar(g1v[:, :, 0:1], g5[:, :, 0:1], 1.0,
                                        None, Alu.mult)
                q0, q1 = img4v[:, ROW0], img4v[:, ROW1]
                w0 = fsh4v[:, ROW0]
                ve = stnpool.tile([112, 448], dt.float32, tag="nx2",
                                  name=f"ve_{s}")
                stt(ve[:, 0:224], q1, -1.0, q0, Alu.mult, Alu.add)
                stt(ve[:, 224:448], w0, -1.0, q1, Alu.mult, Alu.add)
                mb2 = mapbf[2][:].rearrange("p (r c) -> p r c", r=2)
                stt(mb2[:, 0], g1v[:, 0], 1.0, ve[:, 0:224], Alu.mult,
                    Alu.add)
                stt(mb2[:, 1], g1v[:, 1], 1.0, ve[:, 224:448], Alu.mult,
                    Alu.add)

                # table DMAs to the h=0 group; one h=1 dup copy per sample
                for m in range(4):
                    p0 = 32 * s + m
                    eng = nc.sync if m % 2 == 0 else nc.scalar
                    eng.dma_start(
                        tabs[p0:p0 + 1, :].rearrange("p (a c) -> p a c",
                                                     a=112),
                        mapbf[m][:].unsqueeze(1))
                nc.scalar.dma_start(tabs[32 * s + 16:32 * s + 20, :],
                                    tabs[32 * s:32 * s + 4, :])

            if DEBUG:
                nc.sync.dma_start(dbg_idx.ap()[:], idxt[:])
                nc.sync.dma_start(dbg_tabs.ap()[:], tabs[:])

            pre.close()
            post = ExitStack()
            gopool = post.enter_context(tc.tile_pool(name="gop", bufs=1))

            # ---------- gathers + repacks ----------
            # pairs: col = m*2048 + s*512 + 2c + e  (payload col = 256s + c)
            pairs = bigpool.tile([128, 16 * 512], dt.bfloat16, tag="pairs")
            npart = NIC // 256
            for call in range(NCALL):
                gout = gopool.tile([128, 2 * NIC], dt.bfloat16, tag="gout",
                                   name=f"gout{call}")
                nc.gpsimd.ap_gather(
                    gout[:].rearrange("p (k e) -> p k e", e=2),
                    tabs[:].rearrange("p (k e) -> p k e", e=2),
                    idxt[:, (NIC // 16) * call:(NIC // 16) * (call + 1)],
                    channels=128, num_elems=NE, d=2, num_idxs=NIC)
                for s in range(SPC):
                    for m in range(4):
                        blk = m * 2048 + s * 512
                        for h in range(2):
                            p0 = 64 * h + npart * call
                            eng = nc.scalar if (m + h) % 2 == 0 else nc.sync
                            eng.dma_start(
                                pairs[p0:p0 + npart, blk:blk + 512]
                                .unsqueeze(1),
                                gout[32 * s + 16 * h + m:
                                     32 * s + 16 * h + m + 1, :]
                                .rearrange("p (a c) -> p a c", a=npart))

            if DEBUG:
                nc.sync.dma_start(dbg_pairs.ap()[:], pairs[:])

            # ---------- select + losses (4-sample-wide) ----------
            sel = {}
            for m in range(4):
                pv2 = pairs[:, 2048 * m:2048 * m + 2048].rearrange(
                    "p (n e) -> p n e", e=2)
                nc.vector.copy_predicated(pv2[:, :, 0], par_b[:], pv2[:, :, 1])
                sel[m] = pv2[:, :, 0]

            b1 = gopool.tile([128, 1024], dt.float32, tag="b1", name="b1")
            b2 = gopool.tile([128, 1024], dt.float32, tag="b2", name="b2")

            # acc cols: 0 sq, 1 fw, 2 front, 3 back, 4 lw, 5 term
            stt(b1[:], d_b[:], 1.0, d_b[:], Alu.mult, Alu.mult,
                accum=acc[:, 0:1])
            nc.vector.tensor_scalar(b1[:], fw_b[:], 1.0, 0.0, Alu.mult,
                                    Alu.add, accum_out=acc[:, 1:2])
            nc.vector.tensor_scalar(b1[:], lw_b[:], 1.0, 0.0, Alu.mult,
                                    Alu.add, accum_out=acc[:, 4:5])

            stt(b1[:], d_b[:], 1.0, sel[0], Alu.mult, Alu.add)
            nc.scalar.activation(b1[:], b1[:], Act.Abs)
            stt(b2[:], b1[:], 1.0, fw_b[:], Alu.mult, Alu.mult,
                accum=acc[:, 2:3])

            nc.vector.tensor_scalar(b2[:], fw_b[:], -1.0, 1.0, Alu.mult,
                                    Alu.add)
            stt(b1[:], d_b[:], 1.0, sel[1], Alu.mult, Alu.add)
            nc.scalar.activation(b1[:], b1[:], Act.Abs)
            stt(b1[:], b1[:], 1.0, b2[:], Alu.mult, Alu.mult,
                accum=acc[:, 3:4])

            # laplacian: s' = gP*(24.5*scale)/z + gG (= 2x ref; host /4)
            stt(b2[:], sel[3], 1.0, rr_b[:], Alu.mult, Alu.mult)
            stt(b1[:], b2[:], scal[:, 49:50], sel[2], Alu.mult, Alu.add)
            nc.scalar.activation(b2[:], b1[:], Act.Square)
            stt(b1[:], b2[:], 1.0, lw_b[:], Alu.mult, Alu.mult,
                accum=acc[:, 5:6])

            nc.sync.dma_start(acc_d.ap()[:], acc[:])
            post.close()

    nc.compile()
    return nc


def _get_nc():
    if "nc" not in _cache:
        _cache["nc"] = build_nc()
    return _cache["nc"]


def kernel(gt_points, gt_values, gt_gradients, mc_image, gt_transmat, scale,
           base_values, pred_disp):
    nc = _get_nc()
    gt_points = _f32(gt_points)
    gt_values = _f32(gt_values)
    gt_gradients = _f32(gt_gradients)
    mc45 = _f32(mc_image[:, 4:6])
    gt_transmat = _f32(gt_transmat)
    scale = _f32(scale)
    base_values = _f32(base_values)
    pred_disp = _f32(pred_disp)

    in_maps = []
    for c in range(NCORES):
        sl = slice(SPC * c, SPC * (c + 1))
        tms = np.zeros((1, 49), np.float32)
        tms[0, :48] = gt_transmat[sl].reshape(-1)
        tms[0, 48] = scale[0]
        in_maps.append({
            "pts": _f32(gt_points[sl]),
            "grd": _f32(gt_gradients[sl]),
            "gtv": _f32(gt_values[sl, :, 0]),
            "bsv": _f32(base_values[sl, :, 0]),
            "pdp": _f32(pred_disp[sl]),
            "mc45": _f32(mc45[sl]),
            "tms": tms,
        })

    res = run_bass_kernel_spmd(nc, in_maps, core_ids=list(range(NCORES)))

    sq = fwsum = fa = ba = lwsum = tm = 0.0
    for c in range(NCORES):
        a = res.results[c]["acc"].astype(np.float64)
        sq += a[:, 0].sum()
        fwsum += a[:, 1].sum()
        fa += a[:, 2].sum()
        ba += a[:, 3].sum()
        lwsum += a[:, 4].sum()
        tm += a[:, 5].sum()

    tot = float(B * N)
    loss_base = sq / tot
    loss_front = fa / fwsum
    loss_back = ba / (tot - fwsum)
    loss_sdf = 0.5 * (loss_front + loss_back)
    loss_lap = tm / (4.0 * lwsum)
    return np.array([loss_base, loss_sdf, loss_lap], dtype=np.float32)
